# revision 2
# baseline (speedup 1.0000x reference)
"""Trainium2 Bass kernel for nn_Demolition_splitweight_Conv2d.

Computation (per batch element b, one NeuronCore each):
    out[o, p] = (1/(127*Q)) * sum_k wvec[k] * sum_c round(Q*(conv3x3(x[c]; w[k,c,o]) + b[k,c,o]))
with Q = 12.5, wvec = [-128, 1, 2, 4, 8, 16, 32, 64].

Stage 1 (unchanged from the 25702ns baseline): fp16 single-term products +
fp32 magic-number rounding inside the TensorEngine accumulation; per input
channel 12 contraction rows [9 taps, bias, +M, -M] (M = 3072*4096), 2
channels per 32-row PE segment (HW-probed sequential-within-segment
accumulation), 8 channels/pass, 4 chained passes x 2 out-halves per pixel
block -> P[m][(k,ol), px] = channel-summed rounded ints.

Stage 2 (new): bit recombination as fp8e4 DoubleRow matmuls at 0.5
cycles/col -- half the fp16 cost. P holds ints up to ~+-150, beyond e4m3's
exact-int range (16), so each P half is Dekker-split on the copy engines:
Ah = fp8(P) (Act copy, RNE), Al = P - Ah (DVE scalar_tensor_tensor); both
are exact in e4m3 (the RNE residual of an int <= 256 is an int <= 8). One
DoubleRow matmul per out-half with pair dim = (Ah, Al) and gmat pair slots
duplicated reconstructs sum_k WVEC[k]*P exactly (gmat entries are +-pow2,
e4m3-exact). R = un-scaled integer results; the host applies 1/(127*Q).
HW-verified: rel err 0.00928, identical to the baseline.

The last F16_LAST=3 (small) blocks instead use fp16 A + two plain matmuls:
at the tail the Dekker chain (P -> Ah -> Al, 5 PSUM-reads/block on 2
engines) cannot hide behind the tiny stage-1 blocks, and plain fp16 keeps
the final copy->DMA chain short. Stage-2 for Dekker blocks is emitted
S2_LAG=3 blocks late so the PE reaches it well after the copy chain lands
(lag 1 for the light fp16 tail to spread the osb DMAs).

Sim-swept schedule/pools (TimelineSim == the graded metric): blocks ramp
3,3,4,4,5,7 then 8s with a 6,6,2 tail; psP=5/psR=3 PSUM banks; outp=5
avoids the osb-pool back-pressure (osb tiles are freed only by DMA
completion +900ns sem prop). 30 dep-light warmup matmuls ramp the PE
p-state (2.4GHz needs 3us of continuous busy) under the boot DMA; real
work starts at the boot-DMA semaphore (~4.6us: preamble + HWDGE gen 625 +
DGE delay 650 + transfer 1337 + sem prop 900 -- feed-bound, a chunked boot
was tried and starves block 1). Measured: 24685 ns (TimelineSim), rel err
0.00928 on HW (baseline: 25702 ns / 0.00928; bf16 3-term kernel: 44011).
"""

import numpy as np
import ml_dtypes

import concourse.bass as bass
import concourse.mybir as mybir
from concourse.ap import AP
from concourse.tile import TileContext
from concourse.bass_utils import run_bass_kernel_spmd

# problem dims (hardcoded per the task contract)
B, C, OUT, H, W = 8, 32, 32, 64, 64
KBITS = 8
Q = 12.5
WVEC = np.array([-128, 1, 2, 4, 8, 16, 32, 64], np.float32)
SCALE = float(1.0 / (127.0 * Q))

PW = 66            # padded width  (1 + 64 + 1)
PH = 66            # padded height
PSZ = PH * PW      # 4356
NG = 4             # stage-1 passes: 4 x (4 segments x 2 channels)
KC = 120           # contraction rows per pass (last segment unpadded)
RPC = 12           # contraction rows per channel

BLK_ROWS = [3, 3, 4, 4, 5, 7, 8, 8, 8, 6, 6, 2]   # image rows per block
NPB = len(BLK_ROWS)
BLK_R0 = np.cumsum([0] + BLK_ROWS).tolist()        # first image row
NDEN = 512         # max dense moving dim: one PSUM bank exactly
PADW = 64 * PW + 134  # host pad buffer width

# engine policy: "Rpb on DVE?" / "A1 on DVE for last k blocks"
R_ON_DVE = lambda pb: (pb % 2 == 0) and pb < NPB - 3
A1_ON_DVE_LAST = 1
SPLIT_S2_LAST = 0   # (subsumed by F16_LAST; kept for sweeps)
PAIR_START = None   # explicit osb pair starts (list of bool) or None
S2_LAG = 3          # emit stage2(pb) after stage1(pb + S2_LAG)
F16_LAST = 3        # last k blocks: fp16 A + 2 plain matmuls (light tail)
NWARM = 30          # PE p-state warmup matmuls (128 cols each)
WCW = 2 * NG * 128  # weight cols: 8 stage-1 lhsT blocks (gmat separate, fp8)
GMW = 128           # gmat pair cols: [gm0|gm0|gm1|gm1]
F16 = mybir.dt.float16
F32 = mybir.dt.float32
F8 = mybir.dt.float8e4
NP8 = ml_dtypes.float8_e4m3

_cache = {}


def _f16(a):
    return np.asarray(a, np.float32).astype(np.float16)


def _row_of(c, j):
    """Contraction row (pass g, partition w) for channel c, intra row j."""
    g, r = divmod(c, 8)
    s, t = divmod(r, 2)
    return g, s * 32 + t * RPC + j


def _prep_weights(weight, bias):
    """Stage-1 lhsT blocks [128, WCW] fp16 and gmat pairs [128, 64] fp8."""
    qw = _f16(Q * weight.astype(np.float32)).reshape(KBITS, C, 2, 16, 9)
    qb = _f16(Q * bias.astype(np.float32)).reshape(KBITS, C, 2, 16)

    Wt = np.zeros((NG, KC, 2, 128), np.float16)
    for c in range(C):
        for j in range(9):
            g, w = _row_of(c, j)
            Wt[g, w] = qw[:, c, :, :, j].transpose(1, 0, 2).reshape(2, 128)
        g, w = _row_of(c, 9)
        Wt[g, w] = qb[:, c].transpose(1, 0, 2).reshape(2, 128)
        g, w = _row_of(c, 10)
        Wt[g, w] = np.float16(3072.0)
        g, w = _row_of(c, 11)
        Wt[g, w] = np.float16(-3072.0)

    wc = np.zeros((128, WCW + 64), np.float16)
    for g in range(NG):
        for m in range(2):
            wc[:KC, (g * 2 + m) * 128:(g * 2 + m + 1) * 128] = Wt[g, :, m, :]

    j = np.arange(128)
    k_of, ol_of = j // 16, j % 16
    gm0 = np.zeros((128, 32), np.float32)
    gm1 = np.zeros((128, 32), np.float32)
    gm0[j, ol_of] = WVEC[k_of]        # m=0 half -> out rows 0..15
    gm1[j, 16 + ol_of] = WVEC[k_of]   # m=1 half -> out rows 16..31
    # fp16 gmat halves ride in the boot tile (pow2 values, fp16-exact)
    wc[:, WCW:WCW + 32] = gm0.astype(np.float16)
    wc[:, WCW + 32:WCW + 64] = gm1.astype(np.float16)
    # DoubleRow pair dim = (Ah, Al): both slots use the same gm half
    gmat = np.concatenate([gm0, gm0, gm1, gm1], axis=1)
    return wc, gmat.astype(NP8)


def _build_xrep(x):
    """Host REP: per-block [B, KC, NG*nr*PW] fp16 slices."""
    xh = _f16(x)
    xpad = np.zeros((B, C, PADW), np.float16)
    xpad[:, :, :PSZ].reshape(B, C, PH, PW)[:, :, 1:H + 1, 1:W + 1] = xh

    rep = np.zeros((B, NG, KC, H * PW), np.float16)
    for j in range(9):
        off = (j // 3) * PW + (j % 3)
        for c in range(C):
            g, w = _row_of(c, j)
            rep[:, g, w, :] = xpad[:, c, off:off + H * PW]
    for c in range(C):
        g, w = _row_of(c, 9)
        rep[:, g, w, :] = np.float16(1.0)
        for j in (10, 11):
            g, w = _row_of(c, j)
            rep[:, g, w, :] = np.float16(4096.0)
    parts = []
    for pb in range(NPB):
        r0, nr = BLK_R0[pb], BLK_ROWS[pb]
        sl = rep[:, :, :, r0 * PW:(r0 + nr) * PW]           # [B, NG, KC, w]
        parts.append(sl.transpose(0, 2, 1, 3).reshape(B, KC, NG * nr * PW))
    return parts


def _split_multiwaits(nc):
    """This container's walrus allows one sync-wait per instruction; move
    extras onto preceding same-engine NoOps."""
    for bb in nc.main_func.blocks:
        insts = bb.instructions
        i = 0
        while i < len(insts):
            ins = insts[i]
            si = getattr(ins, "sync_info", None)
            if si is not None and si.on_wait is not None and len(si.on_wait) > 1:
                waits = list(si.on_wait)
                nops = []
                for j, w in enumerate(waits[:-1]):
                    nop = mybir.InstNoOp(name=f"{ins.name}-wsplit{j}", ins=[], outs=[])
                    nop.engine = ins.engine
                    nop.sync_info = mybir.SyncInfo(on_wait=[w], on_update=[])
                    nops.append(nop)
                si.on_wait = [waits[-1]]
                ins.sync_info = si
                for j, nop in enumerate(nops):
                    insts.insert(i + j, nop)
                i += len(nops)
            i += 1


def _build_nc():
    bootw = WCW + 64 + NG * BLK_ROWS[0] * PW  # boot: weights+gm16+block 0
    xrepw = NG * (H - BLK_ROWS[0]) * PW      # xrep dram: blocks 1..NPB-1

    nc = bass.Bass()
    boot_d = nc.dram_tensor("boot", [128, bootw], F16, kind="ExternalInput")
    gm_d = nc.dram_tensor("gmat", [128, GMW], F8, kind="ExternalInput")
    xrep_d = nc.dram_tensor("xrep", [KC, xrepw], F16, kind="ExternalInput")
    out_d = nc.dram_tensor("out", [OUT, H * W], F16, kind="ExternalOutput")

    with TileContext(nc) as tc:
        with (
            tc.tile_pool(name="const", bufs=1) as cpool,
            tc.tile_pool(name="blk", bufs=1) as bpool,
            tc.tile_pool(name="work", bufs=6) as wpool,
            tc.tile_pool(name="outp", bufs=5) as opool,
            tc.tile_pool(name="psP", bufs=5, space="PSUM") as psP,
            tc.tile_pool(name="psR", bufs=3, space="PSUM") as psR,
        ):
            # warmup: PE p-state ramp on a zero tile
            warm = cpool.tile([128, 128], F16, tag="warm")
            nc.gpsimd.memset(warm[:, :], 0.0)
            warm_ps = psP.tile([128, NDEN], F32, tag="P", name="warmps")
            for wi in range(NWARM):
                nc.tensor.matmul(warm_ps[:, :128], warm[:, :], warm[:, :],
                                 start=True, stop=True)

            boot = cpool.tile([128, bootw], F16, tag="boot")
            nc.sync.dma_start(out=boot[:, :], in_=boot_d[:, :])
            gm = cpool.tile([128, GMW], F8, tag="gm")
            nc.sync.dma_start(out=gm[:, :], in_=gm_d[:, :])

            blk = [None] * NPB
            off_x = 0
            for pb in range(1, NPB):
                w = NG * BLK_ROWS[pb] * PW
                blk[pb] = bpool.tile([KC, w], F16, tag=f"blk{pb}",
                                     name=f"blk{pb}")
                src = AP(tensor=xrep_d, offset=off_x, ap=[[xrepw, KC], [1, w]])
                dst = AP(tensor=blk[pb].tensor, offset=blk[pb].offset,
                         ap=[[w, KC], [1, w]])
                nc.sync.dma_start(out=dst, in_=src)
                off_x += w

            def stage1(pb):
                nr = BLK_ROWS[pb]
                n = nr * W
                gw = nr * PW
                P = [psP.tile([128, NDEN], F32, tag="P", name=f"P{pb}_{m}")
                     for m in range(2)]
                # last two blocks run m-outer so P0's chain (and its copy)
                # completes 4 matmuls earlier, hiding copy latency in the
                # tail; earlier blocks stay m-inner
                gm_order = [(g, m) for m in range(2) for g in range(NG)]
                for g, m in gm_order:
                    if pb == 0:
                        t = boot
                        off, pitch = boot.offset + WCW + 64 + g * gw, bootw
                    else:
                        t = blk[pb]
                        off, pitch = t.offset + g * gw, NG * gw
                    rhs = AP(tensor=t.tensor, offset=off,
                             ap=[[pitch, KC], [PW, nr], [1, W]])
                    lt = boot[0:KC, (g * 2 + m) * 128:(g * 2 + m + 1) * 128]
                    nc.tensor.matmul(P[m][:, :n], lt, rhs,
                                     start=(g == 0), stop=(g == NG - 1))
                if pb >= NPB - F16_LAST:
                    # light tail path: fp16 A (ints <= 2048 exact), 2 copies
                    A = wpool.tile([128, 2 * NDEN], F16, tag="A16",
                                   name=f"A{pb}")
                    nc.scalar.copy(A[:, 0:n], P[0][:, :n])
                    nc.vector.tensor_copy(A[:, NDEN:NDEN + n], P[1][:, :n])
                    return A
                # A: fp8 tile [Ah0|Ah1|Al0|Al1]. P values are channel-summed
                # ints (|P| <~ 150): Dekker-split P = Ah + Al, both exact in
                # e4m3 (RNE residual of an int <= 256 is an int <= 8).
                A = wpool.tile([128, 4 * NDEN], F8, tag="A", name=f"A{pb}")
                nc.scalar.copy(A[:, 0:n], P[0][:, :n])
                nc.vector.scalar_tensor_tensor(
                    A[:, 2 * NDEN:2 * NDEN + n], P[0][:, :n], 0.0, A[:, 0:n],
                    mybir.AluOpType.add, mybir.AluOpType.subtract)
                nc.scalar.copy(A[:, NDEN:NDEN + n], P[1][:, :n])
                nc.vector.scalar_tensor_tensor(
                    A[:, 3 * NDEN:3 * NDEN + n], P[1][:, :n], 0.0,
                    A[:, NDEN:NDEN + n],
                    mybir.AluOpType.add, mybir.AluOpType.subtract)
                return A

            osb = None
            osb_off = 0
            osb_pb0 = 0
            pair_start = (list(PAIR_START) if PAIR_START is not None else
                          [True] + [(pb % 2) == (NPB % 2)
                                    for pb in range(1, NPB)])

            def stage2(pb, A):
                nonlocal osb, osb_off, osb_pb0
                nr = BLK_ROWS[pb]
                n = nr * W
                R = psR.tile([32, NDEN], F32, tag="R", name=f"R{pb}")
                if pb >= NPB - F16_LAST:
                    # fp16 A: two plain matmuls with the boot-resident gmat
                    nc.tensor.matmul(R[:, :n], boot[:, WCW:WCW + 32],
                                     A[:, 0:n], start=True, stop=False)
                    nc.tensor.matmul(R[:, :n], boot[:, WCW + 32:WCW + 64],
                                     A[:, NDEN:NDEN + n],
                                     start=False, stop=True)
                elif pb >= NPB - SPLIT_S2_LAST:
                    # four plain fp8 matmuls: each needs only one A quarter
                    for lo, ao in ((0, 0), (0, 2 * NDEN), (64, NDEN),
                                   (64, 3 * NDEN)):
                        nc.tensor.matmul(R[:, :n], gm[:, lo:lo + 32],
                                         A[:, ao:ao + n],
                                         start=(ao == 0), stop=(lo == 64 and
                                                                ao >= 3 * NDEN))
                else:
                    # two DoubleRow matmuls; pair dim = (Ah, Al) per m-half,
                    # so the m0 matmul only needs the m0 copies
                    for m in range(2):
                        lt = AP(tensor=gm.tensor, offset=gm.offset + 64 * m,
                                ap=[[GMW, 128], [32, 2], [1, 32]])
                        rhs = AP(tensor=A.tensor, offset=A.offset + m * NDEN,
                                 ap=[[4 * NDEN, 128], [2 * NDEN, 2], [1, n]])
                        nc.tensor.matmul(R[:, :n], lt, rhs, start=(m == 0),
                                         stop=(m == 1),
                                         perf_mode=mybir.MatmulPerfMode.DoubleRow)
                if pair_start[pb]:
                    osb = opool.tile([32, 2 * NDEN], F16, tag="osb",
                                     name=f"osb{pb}")
                    osb_off = 0
                    osb_pb0 = pb
                out_ap = AP(tensor=osb.tensor, offset=osb.offset + osb_off,
                            ap=[[2 * NDEN, 32], [1, n]])
                if R_ON_DVE(pb):
                    nc.vector.tensor_copy(out_ap, R[:, :n])
                else:
                    nc.scalar.copy(out_ap, R[:, :n])
                osb_off += n
                if pb == NPB - 1 or pair_start[pb + 1]:
                    pb0 = osb_pb0
                    nrows = sum(BLK_ROWS[pb0:pb + 1])
                    dst = AP(tensor=out_d, offset=BLK_R0[pb0] * W,
                             ap=[[H * W, OUT], [1, nrows * W]])
                    src = AP(tensor=osb.tensor, offset=osb.offset,
                             ap=[[2 * NDEN, 32], [1, nrows * W]])
                    nc.sync.dma_start(out=dst, in_=src)

            def lag_of(q):
                return 1 if q >= NPB - F16_LAST else S2_LAG

            As = []
            nxt = 0
            for pb in range(NPB):
                As.append(stage1(pb))
                while nxt < NPB and nxt <= pb - lag_of(nxt):
                    stage2(nxt, As[nxt])
                    nxt += 1
            while nxt < NPB:
                stage2(nxt, As[nxt])
                nxt += 1

    _split_multiwaits(nc)
    return nc


def kernel(x, weight, bias):
    x = np.asarray(x, np.float32)
    weight = np.asarray(weight, np.float32)
    bias = np.asarray(bias, np.float32)

    parts = _build_xrep(x)
    wc, gmat = _prep_weights(weight, bias)
    bootw = WCW + 64 + NG * BLK_ROWS[0] * PW

    if "nc" not in _cache:
        _cache["nc"] = _build_nc()
    nc = _cache["nc"]

    in_maps = []
    for b in range(B):
        boot = np.zeros((128, bootw), np.float16)
        boot[:, :WCW + 64] = wc
        boot[:KC, WCW + 64:] = parts[0][b]
        xr = np.concatenate([p[b] for p in parts[1:]], axis=1)
        in_maps.append({"boot": boot, "gmat": gmat, "xrep": xr})
    res = run_bass_kernel_spmd(nc, in_maps, core_ids=list(range(B)))
    out = np.stack([np.asarray(r["out"], np.float32) for r in res.results])
    return (out * SCALE).reshape(B, OUT, H, W).astype(np.float32)


# revision 4
# speedup vs baseline: 1.0036x; 1.0036x over previous
"""Trainium2 Bass kernel for nn_Demolition_splitweight_Conv2d.

Computation (per batch element b, one NeuronCore each):
    out[o, p] = (1/(127*Q)) * sum_k wvec[k] * sum_c round(Q*(conv3x3(x[c]; w[k,c,o]) + b[k,c,o]))
with Q = 12.5, wvec = [-128, 1, 2, 4, 8, 16, 32, 64].

Stage 1 (unchanged from the 25702ns baseline): fp16 single-term products +
fp32 magic-number rounding inside the TensorEngine accumulation; per input
channel 12 contraction rows [9 taps, bias, +M, -M] (M = 3072*4096), 2
channels per 32-row PE segment (HW-probed sequential-within-segment
accumulation), 8 channels/pass, 4 chained passes x 2 out-halves per pixel
block -> P[m][(k,ol), px] = channel-summed rounded ints.

Stage 2 (new): bit recombination as fp8e4 DoubleRow matmuls at 0.5
cycles/col -- half the fp16 cost. P holds ints up to ~+-150, beyond e4m3's
exact-int range (16), so each P half is Dekker-split on the copy engines:
Ah = fp8(P) (Act copy, RNE), Al = P - Ah (DVE scalar_tensor_tensor); both
are exact in e4m3 (the RNE residual of an int <= 256 is an int <= 8). One
DoubleRow matmul per out-half with pair dim = (Ah, Al) and gmat pair slots
duplicated reconstructs sum_k WVEC[k]*P exactly (gmat entries are +-pow2,
e4m3-exact). R = un-scaled integer results; the host applies 1/(127*Q).
HW-verified: rel err 0.00928, identical to the baseline.

The last F16_LAST=3 (small) blocks instead use fp16 A + two plain matmuls:
at the tail the Dekker chain (P -> Ah -> Al, 5 PSUM-reads/block on 2
engines) cannot hide behind the tiny stage-1 blocks, and plain fp16 keeps
the final copy->DMA chain short. Stage-2 for Dekker blocks is emitted
S2_LAG=3 blocks late so the PE reaches it well after the copy chain lands
(lag 1 for the light fp16 tail to spread the osb DMAs).

Sim-swept schedule/pools (TimelineSim == the graded metric): blocks ramp
3,3,4,4,5,7 then 8s with a 6,6,2 tail; psP=5/psR=3 PSUM banks; outp=5
avoids the osb-pool back-pressure (osb tiles are freed only by DMA
completion +900ns sem prop). 30 dep-light warmup matmuls ramp the PE
p-state (2.4GHz needs 3us of continuous busy) under the boot DMA; real
work starts at the boot-DMA semaphore (~4.6us: preamble + HWDGE gen 625 +
DGE delay 650 + transfer 1337 + sem prop 900 -- feed-bound, a chunked boot
was tried and starves block 1). Measured: 24685 ns (TimelineSim), rel err
0.00928 on HW (baseline: 25702 ns / 0.00928; bf16 3-term kernel: 44011).
"""

import numpy as np
import ml_dtypes

import concourse.bass as bass
import concourse.mybir as mybir
from concourse.ap import AP
from concourse.tile import TileContext
from concourse.bass_utils import run_bass_kernel_spmd

# problem dims (hardcoded per the task contract)
B, C, OUT, H, W = 8, 32, 32, 64, 64
KBITS = 8
Q = 12.5
WVEC = np.array([-128, 1, 2, 4, 8, 16, 32, 64], np.float32)
SCALE = float(1.0 / (127.0 * Q))

PW = 66            # padded width  (1 + 64 + 1)
PH = 66            # padded height
PSZ = PH * PW      # 4356
NG = 4             # stage-1 passes: 4 x (4 segments x 2 channels)
KC = 120           # contraction rows per pass (last segment unpadded)
RPC = 12           # contraction rows per channel

BLK_ROWS = [3, 3, 4, 4, 5, 7, 8, 8, 8, 6, 6, 2]   # image rows per block
NPB = len(BLK_ROWS)
BLK_R0 = np.cumsum([0] + BLK_ROWS).tolist()        # first image row
NDEN = 512         # max dense moving dim: one PSUM bank exactly
PADW = 64 * PW + 134  # host pad buffer width

# engine policy: "Rpb on DVE?" / "A1 on DVE for last k blocks"
R_ON_DVE = lambda pb: ((pb % 2 == 0) and pb < NPB - 3) or pb >= NPB - 2
A1_ON_DVE_LAST = 1
SPLIT_S2_LAST = 0   # (subsumed by F16_LAST; kept for sweeps)
PAIR_START = None   # explicit osb pair starts (list of bool) or None
S2_LAG = 3          # emit stage2(pb) after stage1(pb + S2_LAG)
F16_LAST = 3        # last k blocks: fp16 A + 2 plain matmuls (light tail)
NWARM = 30          # PE p-state warmup matmuls (128 cols each)
WCW = 2 * NG * 128  # weight cols: 8 stage-1 lhsT blocks (gmat separate, fp8)
GMW = 128           # gmat pair cols: [gm0|gm0|gm1|gm1]
F16 = mybir.dt.float16
F32 = mybir.dt.float32
F8 = mybir.dt.float8e4
NP8 = ml_dtypes.float8_e4m3

_cache = {}


def _f16(a):
    return np.asarray(a, np.float32).astype(np.float16)


def _row_of(c, j):
    """Contraction row (pass g, partition w) for channel c, intra row j."""
    g, r = divmod(c, 8)
    s, t = divmod(r, 2)
    return g, s * 32 + t * RPC + j


def _prep_weights(weight, bias):
    """Stage-1 lhsT blocks [128, WCW] fp16 and gmat pairs [128, 64] fp8."""
    qw = _f16(Q * weight.astype(np.float32)).reshape(KBITS, C, 2, 16, 9)
    qb = _f16(Q * bias.astype(np.float32)).reshape(KBITS, C, 2, 16)

    Wt = np.zeros((NG, KC, 2, 128), np.float16)
    for c in range(C):
        for j in range(9):
            g, w = _row_of(c, j)
            Wt[g, w] = qw[:, c, :, :, j].transpose(1, 0, 2).reshape(2, 128)
        g, w = _row_of(c, 9)
        Wt[g, w] = qb[:, c].transpose(1, 0, 2).reshape(2, 128)
        g, w = _row_of(c, 10)
        Wt[g, w] = np.float16(3072.0)
        g, w = _row_of(c, 11)
        Wt[g, w] = np.float16(-3072.0)

    wc = np.zeros((128, WCW + 64), np.float16)
    for g in range(NG):
        for m in range(2):
            wc[:KC, (g * 2 + m) * 128:(g * 2 + m + 1) * 128] = Wt[g, :, m, :]

    j = np.arange(128)
    k_of, ol_of = j // 16, j % 16
    gm0 = np.zeros((128, 32), np.float32)
    gm1 = np.zeros((128, 32), np.float32)
    gm0[j, ol_of] = WVEC[k_of]        # m=0 half -> out rows 0..15
    gm1[j, 16 + ol_of] = WVEC[k_of]   # m=1 half -> out rows 16..31
    # fp16 gmat halves ride in the boot tile (pow2 values, fp16-exact)
    wc[:, WCW:WCW + 32] = gm0.astype(np.float16)
    wc[:, WCW + 32:WCW + 64] = gm1.astype(np.float16)
    # DoubleRow pair dim = (Ah, Al): both slots use the same gm half
    gmat = np.concatenate([gm0, gm0, gm1, gm1], axis=1)
    return wc, gmat.astype(NP8)


def _build_xrep(x):
    """Host REP: per-block [B, KC, NG*nr*PW] fp16 slices."""
    xh = _f16(x)
    xpad = np.zeros((B, C, PADW), np.float16)
    xpad[:, :, :PSZ].reshape(B, C, PH, PW)[:, :, 1:H + 1, 1:W + 1] = xh

    rep = np.zeros((B, NG, KC, H * PW), np.float16)
    for j in range(9):
        off = (j // 3) * PW + (j % 3)
        for c in range(C):
            g, w = _row_of(c, j)
            rep[:, g, w, :] = xpad[:, c, off:off + H * PW]
    for c in range(C):
        g, w = _row_of(c, 9)
        rep[:, g, w, :] = np.float16(1.0)
        for j in (10, 11):
            g, w = _row_of(c, j)
            rep[:, g, w, :] = np.float16(4096.0)
    parts = []
    for pb in range(NPB):
        r0, nr = BLK_R0[pb], BLK_ROWS[pb]
        sl = rep[:, :, :, r0 * PW:(r0 + nr) * PW]           # [B, NG, KC, w]
        parts.append(sl.transpose(0, 2, 1, 3).reshape(B, KC, NG * nr * PW))
    return parts


def _split_multiwaits(nc):
    """This container's walrus allows one sync-wait per instruction; move
    extras onto preceding same-engine NoOps."""
    for bb in nc.main_func.blocks:
        insts = bb.instructions
        i = 0
        while i < len(insts):
            ins = insts[i]
            si = getattr(ins, "sync_info", None)
            if si is not None and si.on_wait is not None and len(si.on_wait) > 1:
                waits = list(si.on_wait)
                nops = []
                for j, w in enumerate(waits[:-1]):
                    nop = mybir.InstNoOp(name=f"{ins.name}-wsplit{j}", ins=[], outs=[])
                    nop.engine = ins.engine
                    nop.sync_info = mybir.SyncInfo(on_wait=[w], on_update=[])
                    nops.append(nop)
                si.on_wait = [waits[-1]]
                ins.sync_info = si
                for j, nop in enumerate(nops):
                    insts.insert(i + j, nop)
                i += len(nops)
            i += 1


def _build_nc():
    bootw = WCW + 64 + NG * BLK_ROWS[0] * PW  # boot: weights+gm16+block 0
    xrepw = NG * (H - BLK_ROWS[0]) * PW      # xrep dram: blocks 1..NPB-1

    nc = bass.Bass()
    boot_d = nc.dram_tensor("boot", [128, bootw], F16, kind="ExternalInput")
    gm_d = nc.dram_tensor("gmat", [128, GMW], F8, kind="ExternalInput")
    xrep_d = nc.dram_tensor("xrep", [KC, xrepw], F16, kind="ExternalInput")
    out_d = nc.dram_tensor("out", [OUT, H * W], F16, kind="ExternalOutput")

    with TileContext(nc) as tc:
        with (
            tc.tile_pool(name="const", bufs=1) as cpool,
            tc.tile_pool(name="blk", bufs=1) as bpool,
            tc.tile_pool(name="work", bufs=6) as wpool,
            tc.tile_pool(name="outp", bufs=5) as opool,
            tc.tile_pool(name="psP", bufs=5, space="PSUM") as psP,
            tc.tile_pool(name="psR", bufs=3, space="PSUM") as psR,
        ):
            # warmup: PE p-state ramp on a zero tile
            warm = cpool.tile([128, 128], F16, tag="warm")
            nc.gpsimd.memset(warm[:, :], 0.0)
            warm_ps = psP.tile([128, NDEN], F32, tag="P", name="warmps")
            for wi in range(NWARM):
                nc.tensor.matmul(warm_ps[:, :128], warm[:, :], warm[:, :],
                                 start=True, stop=True)

            boot = cpool.tile([128, bootw], F16, tag="boot")
            nc.sync.dma_start(out=boot[:, :], in_=boot_d[:, :])
            gm = cpool.tile([128, GMW], F8, tag="gm")
            nc.sync.dma_start(out=gm[:, :], in_=gm_d[:, :])

            blk = [None] * NPB
            off_x = 0
            for pb in range(1, NPB):
                w = NG * BLK_ROWS[pb] * PW
                blk[pb] = bpool.tile([KC, w], F16, tag=f"blk{pb}",
                                     name=f"blk{pb}")
                src = AP(tensor=xrep_d, offset=off_x, ap=[[xrepw, KC], [1, w]])
                dst = AP(tensor=blk[pb].tensor, offset=blk[pb].offset,
                         ap=[[w, KC], [1, w]])
                nc.sync.dma_start(out=dst, in_=src)
                off_x += w

            def stage1(pb):
                nr = BLK_ROWS[pb]
                n = nr * W
                gw = nr * PW
                P = [psP.tile([128, NDEN], F32, tag="P", name=f"P{pb}_{m}")
                     for m in range(2)]
                # last two blocks run m-outer so P0's chain (and its copy)
                # completes 4 matmuls earlier, hiding copy latency in the
                # tail; earlier blocks stay m-inner
                gm_order = [(g, m) for m in range(2) for g in range(NG)]
                for g, m in gm_order:
                    if pb == 0:
                        t = boot
                        off, pitch = boot.offset + WCW + 64 + g * gw, bootw
                    else:
                        t = blk[pb]
                        off, pitch = t.offset + g * gw, NG * gw
                    rhs = AP(tensor=t.tensor, offset=off,
                             ap=[[pitch, KC], [PW, nr], [1, W]])
                    lt = boot[0:KC, (g * 2 + m) * 128:(g * 2 + m + 1) * 128]
                    nc.tensor.matmul(P[m][:, :n], lt, rhs,
                                     start=(g == 0), stop=(g == NG - 1))
                if pb >= NPB - F16_LAST:
                    # light tail path: fp16 A (ints <= 2048 exact), 2 copies
                    A = wpool.tile([128, 2 * NDEN], F16, tag="A16",
                                   name=f"A{pb}")
                    nc.scalar.copy(A[:, 0:n], P[0][:, :n])
                    nc.vector.tensor_copy(A[:, NDEN:NDEN + n], P[1][:, :n])
                    return A
                # A: fp8 tile [Ah0|Ah1|Al0|Al1]. P values are channel-summed
                # ints (|P| <~ 150): Dekker-split P = Ah + Al, both exact in
                # e4m3 (RNE residual of an int <= 256 is an int <= 8).
                A = wpool.tile([128, 4 * NDEN], F8, tag="A", name=f"A{pb}")
                nc.scalar.copy(A[:, 0:n], P[0][:, :n])
                nc.vector.scalar_tensor_tensor(
                    A[:, 2 * NDEN:2 * NDEN + n], P[0][:, :n], 0.0, A[:, 0:n],
                    mybir.AluOpType.add, mybir.AluOpType.subtract)
                nc.scalar.copy(A[:, NDEN:NDEN + n], P[1][:, :n])
                nc.vector.scalar_tensor_tensor(
                    A[:, 3 * NDEN:3 * NDEN + n], P[1][:, :n], 0.0,
                    A[:, NDEN:NDEN + n],
                    mybir.AluOpType.add, mybir.AluOpType.subtract)
                return A

            osb = None
            osb_off = 0
            osb_pb0 = 0
            pair_start = (list(PAIR_START) if PAIR_START is not None else
                          [True] + [(pb % 2) == (NPB % 2)
                                    for pb in range(1, NPB)])

            def stage2(pb, A):
                nonlocal osb, osb_off, osb_pb0
                nr = BLK_ROWS[pb]
                n = nr * W
                R = psR.tile([32, NDEN], F32, tag="R", name=f"R{pb}")
                if pb >= NPB - F16_LAST:
                    # fp16 A: two plain matmuls with the boot-resident gmat
                    nc.tensor.matmul(R[:, :n], boot[:, WCW:WCW + 32],
                                     A[:, 0:n], start=True, stop=False)
                    nc.tensor.matmul(R[:, :n], boot[:, WCW + 32:WCW + 64],
                                     A[:, NDEN:NDEN + n],
                                     start=False, stop=True)
                elif pb >= NPB - SPLIT_S2_LAST:
                    # four plain fp8 matmuls: each needs only one A quarter
                    for lo, ao in ((0, 0), (0, 2 * NDEN), (64, NDEN),
                                   (64, 3 * NDEN)):
                        nc.tensor.matmul(R[:, :n], gm[:, lo:lo + 32],
                                         A[:, ao:ao + n],
                                         start=(ao == 0), stop=(lo == 64 and
                                                                ao >= 3 * NDEN))
                else:
                    # two DoubleRow matmuls; pair dim = (Ah, Al) per m-half,
                    # so the m0 matmul only needs the m0 copies
                    for m in range(2):
                        lt = AP(tensor=gm.tensor, offset=gm.offset + 64 * m,
                                ap=[[GMW, 128], [32, 2], [1, 32]])
                        rhs = AP(tensor=A.tensor, offset=A.offset + m * NDEN,
                                 ap=[[4 * NDEN, 128], [2 * NDEN, 2], [1, n]])
                        nc.tensor.matmul(R[:, :n], lt, rhs, start=(m == 0),
                                         stop=(m == 1),
                                         perf_mode=mybir.MatmulPerfMode.DoubleRow)
                if pair_start[pb]:
                    osb = opool.tile([32, 2 * NDEN], F16, tag="osb",
                                     name=f"osb{pb}")
                    osb_off = 0
                    osb_pb0 = pb
                out_ap = AP(tensor=osb.tensor, offset=osb.offset + osb_off,
                            ap=[[2 * NDEN, 32], [1, n]])
                if R_ON_DVE(pb):
                    nc.vector.tensor_copy(out_ap, R[:, :n])
                else:
                    nc.scalar.copy(out_ap, R[:, :n])
                osb_off += n
                if pb == NPB - 1 or pair_start[pb + 1]:
                    pb0 = osb_pb0
                    nrows = sum(BLK_ROWS[pb0:pb + 1])
                    dst = AP(tensor=out_d, offset=BLK_R0[pb0] * W,
                             ap=[[H * W, OUT], [1, nrows * W]])
                    src = AP(tensor=osb.tensor, offset=osb.offset,
                             ap=[[2 * NDEN, 32], [1, nrows * W]])
                    nc.sync.dma_start(out=dst, in_=src)

            def lag_of(q):
                if q >= NPB - F16_LAST:
                    return 1
                # last two Dekker blocks: shorter lag so their R copies
                # free psR banks before the tail stage-2s need them (the
                # scheduler reorders around waits, so emitting earlier
                # lets them run at dep-ready time)
                return 2 if q >= NPB - F16_LAST - 2 else S2_LAG

            As = []
            nxt = 0
            for pb in range(NPB):
                As.append(stage1(pb))
                while nxt < NPB and nxt <= pb - lag_of(nxt):
                    stage2(nxt, As[nxt])
                    nxt += 1
            while nxt < NPB:
                stage2(nxt, As[nxt])
                nxt += 1

    _split_multiwaits(nc)
    return nc


def kernel(x, weight, bias):
    x = np.asarray(x, np.float32)
    weight = np.asarray(weight, np.float32)
    bias = np.asarray(bias, np.float32)

    parts = _build_xrep(x)
    wc, gmat = _prep_weights(weight, bias)
    bootw = WCW + 64 + NG * BLK_ROWS[0] * PW

    if "nc" not in _cache:
        _cache["nc"] = _build_nc()
    nc = _cache["nc"]

    in_maps = []
    for b in range(B):
        boot = np.zeros((128, bootw), np.float16)
        boot[:, :WCW + 64] = wc
        boot[:KC, WCW + 64:] = parts[0][b]
        xr = np.concatenate([p[b] for p in parts[1:]], axis=1)
        in_maps.append({"boot": boot, "gmat": gmat, "xrep": xr})
    res = run_bass_kernel_spmd(nc, in_maps, core_ids=list(range(B)))
    out = np.stack([np.asarray(r["out"], np.float32) for r in res.results])
    return (out * SCALE).reshape(B, OUT, H, W).astype(np.float32)


# revision 5
# speedup vs baseline: 1.0064x; 1.0028x over previous
"""Trainium2 Bass kernel for nn_Demolition_splitweight_Conv2d.

Computation (per batch element b, one NeuronCore each):
    out[o, p] = (1/(127*Q)) * sum_k wvec[k] * sum_c round(Q*(conv3x3(x[c]; w[k,c,o]) + b[k,c,o]))
with Q = 12.5, wvec = [-128, 1, 2, 4, 8, 16, 32, 64].

Stage 1 (unchanged from the 25702ns baseline): fp16 single-term products +
fp32 magic-number rounding inside the TensorEngine accumulation; per input
channel 12 contraction rows [9 taps, bias, +M, -M] (M = 3072*4096), 2
channels per 32-row PE segment (HW-probed sequential-within-segment
accumulation), 8 channels/pass, 4 chained passes x 2 out-halves per pixel
block -> P[m][(k,ol), px] = channel-summed rounded ints.

Stage 2 (new): bit recombination as fp8e4 DoubleRow matmuls at 0.5
cycles/col -- half the fp16 cost. P holds ints up to ~+-150, beyond e4m3's
exact-int range (16), so each P half is Dekker-split on the copy engines:
Ah = fp8(P) (Act copy, RNE), Al = P - Ah (DVE scalar_tensor_tensor); both
are exact in e4m3 (the RNE residual of an int <= 256 is an int <= 8). One
DoubleRow matmul per out-half with pair dim = (Ah, Al) and gmat pair slots
duplicated reconstructs sum_k WVEC[k]*P exactly (gmat entries are +-pow2,
e4m3-exact). R = un-scaled integer results; the host applies 1/(127*Q).
HW-verified: rel err 0.00928, identical to the baseline.

The last F16_LAST=3 (small) blocks instead use fp16 A + two plain matmuls:
at the tail the Dekker chain (P -> Ah -> Al, 5 PSUM-reads/block on 2
engines) cannot hide behind the tiny stage-1 blocks, and plain fp16 keeps
the final copy->DMA chain short. Stage-2 for Dekker blocks is emitted
S2_LAG=3 blocks late so the PE reaches it well after the copy chain lands
(lag 1 for the light fp16 tail to spread the osb DMAs).

Sim-swept schedule/pools (TimelineSim == the graded metric): blocks ramp
3,3,4,4,5,7 then 8s with a 7,4,3 tail; psP=5/psR=3 PSUM banks; outp=5
avoids the osb-pool back-pressure (osb tiles are freed only by DMA
completion +900ns sem prop). 30 dep-light warmup matmuls ramp the PE
p-state (2.4GHz needs 3us of continuous busy) under the boot DMA; real
work starts at the boot-DMA semaphore (~4.6us: preamble + HWDGE gen 625 +
DGE delay 650 + transfer 1337 + sem prop 900 -- feed-bound, a chunked boot
was tried and starves block 1). Measured: 24528 ns (TimelineSim), rel err
0.00928 on HW (baseline: 25702 ns / 0.00928; bf16 3-term kernel: 44011).
"""

import numpy as np
import ml_dtypes

import concourse.bass as bass
import concourse.mybir as mybir
from concourse.ap import AP
from concourse.tile import TileContext
from concourse.bass_utils import run_bass_kernel_spmd

# problem dims (hardcoded per the task contract)
B, C, OUT, H, W = 8, 32, 32, 64, 64
KBITS = 8
Q = 12.5
WVEC = np.array([-128, 1, 2, 4, 8, 16, 32, 64], np.float32)
SCALE = float(1.0 / (127.0 * Q))

PW = 66            # padded width  (1 + 64 + 1)
PH = 66            # padded height
PSZ = PH * PW      # 4356
NG = 4             # stage-1 passes: 4 x (4 segments x 2 channels)
KC = 120           # contraction rows per pass (last segment unpadded)
RPC = 12           # contraction rows per channel

BLK_ROWS = [3, 3, 4, 4, 5, 7, 8, 8, 8, 7, 4, 3]   # image rows per block
NPB = len(BLK_ROWS)
BLK_R0 = np.cumsum([0] + BLK_ROWS).tolist()        # first image row
NDEN = 512         # max dense moving dim: one PSUM bank exactly
PADW = 64 * PW + 134  # host pad buffer width

# engine policy: "Rpb on DVE?" / "A1 on DVE for last k blocks"
R_ON_DVE = lambda pb: ((pb % 2 == 0) and pb < NPB - 3) or pb >= NPB - 2
A1_ON_DVE_LAST = 1
SPLIT_S2_LAST = 0   # (subsumed by F16_LAST; kept for sweeps)
PAIR_START = None   # explicit osb pair starts (list of bool) or None
S2_LAG = 3          # emit stage2(pb) after stage1(pb + S2_LAG)
F16_LAST = 3        # last k blocks: fp16 A + 2 plain matmuls (light tail)
NWARM = 30          # PE p-state warmup matmuls (128 cols each)
WCW = 2 * NG * 128  # weight cols: 8 stage-1 lhsT blocks (gmat separate, fp8)
GMW = 128           # gmat pair cols: [gm0|gm0|gm1|gm1]
F16 = mybir.dt.float16
F32 = mybir.dt.float32
F8 = mybir.dt.float8e4
NP8 = ml_dtypes.float8_e4m3

_cache = {}


def _f16(a):
    return np.asarray(a, np.float32).astype(np.float16)


def _row_of(c, j):
    """Contraction row (pass g, partition w) for channel c, intra row j."""
    g, r = divmod(c, 8)
    s, t = divmod(r, 2)
    return g, s * 32 + t * RPC + j


def _prep_weights(weight, bias):
    """Stage-1 lhsT blocks [128, WCW] fp16 and gmat pairs [128, 64] fp8."""
    qw = _f16(Q * weight.astype(np.float32)).reshape(KBITS, C, 2, 16, 9)
    qb = _f16(Q * bias.astype(np.float32)).reshape(KBITS, C, 2, 16)

    Wt = np.zeros((NG, KC, 2, 128), np.float16)
    for c in range(C):
        for j in range(9):
            g, w = _row_of(c, j)
            Wt[g, w] = qw[:, c, :, :, j].transpose(1, 0, 2).reshape(2, 128)
        g, w = _row_of(c, 9)
        Wt[g, w] = qb[:, c].transpose(1, 0, 2).reshape(2, 128)
        g, w = _row_of(c, 10)
        Wt[g, w] = np.float16(3072.0)
        g, w = _row_of(c, 11)
        Wt[g, w] = np.float16(-3072.0)

    wc = np.zeros((128, WCW + 64), np.float16)
    for g in range(NG):
        for m in range(2):
            wc[:KC, (g * 2 + m) * 128:(g * 2 + m + 1) * 128] = Wt[g, :, m, :]

    j = np.arange(128)
    k_of, ol_of = j // 16, j % 16
    gm0 = np.zeros((128, 32), np.float32)
    gm1 = np.zeros((128, 32), np.float32)
    gm0[j, ol_of] = WVEC[k_of]        # m=0 half -> out rows 0..15
    gm1[j, 16 + ol_of] = WVEC[k_of]   # m=1 half -> out rows 16..31
    # fp16 gmat halves ride in the boot tile (pow2 values, fp16-exact)
    wc[:, WCW:WCW + 32] = gm0.astype(np.float16)
    wc[:, WCW + 32:WCW + 64] = gm1.astype(np.float16)
    # DoubleRow pair dim = (Ah, Al): both slots use the same gm half
    gmat = np.concatenate([gm0, gm0, gm1, gm1], axis=1)
    return wc, gmat.astype(NP8)


def _build_xrep(x):
    """Host REP: per-block [B, KC, NG*nr*PW] fp16 slices."""
    xh = _f16(x)
    xpad = np.zeros((B, C, PADW), np.float16)
    xpad[:, :, :PSZ].reshape(B, C, PH, PW)[:, :, 1:H + 1, 1:W + 1] = xh

    rep = np.zeros((B, NG, KC, H * PW), np.float16)
    for j in range(9):
        off = (j // 3) * PW + (j % 3)
        for c in range(C):
            g, w = _row_of(c, j)
            rep[:, g, w, :] = xpad[:, c, off:off + H * PW]
    for c in range(C):
        g, w = _row_of(c, 9)
        rep[:, g, w, :] = np.float16(1.0)
        for j in (10, 11):
            g, w = _row_of(c, j)
            rep[:, g, w, :] = np.float16(4096.0)
    parts = []
    for pb in range(NPB):
        r0, nr = BLK_R0[pb], BLK_ROWS[pb]
        sl = rep[:, :, :, r0 * PW:(r0 + nr) * PW]           # [B, NG, KC, w]
        parts.append(sl.transpose(0, 2, 1, 3).reshape(B, KC, NG * nr * PW))
    return parts


def _split_multiwaits(nc):
    """This container's walrus allows one sync-wait per instruction; move
    extras onto preceding same-engine NoOps."""
    for bb in nc.main_func.blocks:
        insts = bb.instructions
        i = 0
        while i < len(insts):
            ins = insts[i]
            si = getattr(ins, "sync_info", None)
            if si is not None and si.on_wait is not None and len(si.on_wait) > 1:
                waits = list(si.on_wait)
                nops = []
                for j, w in enumerate(waits[:-1]):
                    nop = mybir.InstNoOp(name=f"{ins.name}-wsplit{j}", ins=[], outs=[])
                    nop.engine = ins.engine
                    nop.sync_info = mybir.SyncInfo(on_wait=[w], on_update=[])
                    nops.append(nop)
                si.on_wait = [waits[-1]]
                ins.sync_info = si
                for j, nop in enumerate(nops):
                    insts.insert(i + j, nop)
                i += len(nops)
            i += 1


def _build_nc():
    bootw = WCW + 64 + NG * BLK_ROWS[0] * PW  # boot: weights+gm16+block 0
    xrepw = NG * (H - BLK_ROWS[0]) * PW      # xrep dram: blocks 1..NPB-1

    nc = bass.Bass()
    boot_d = nc.dram_tensor("boot", [128, bootw], F16, kind="ExternalInput")
    gm_d = nc.dram_tensor("gmat", [128, GMW], F8, kind="ExternalInput")
    xrep_d = nc.dram_tensor("xrep", [KC, xrepw], F16, kind="ExternalInput")
    out_d = nc.dram_tensor("out", [OUT, H * W], F16, kind="ExternalOutput")

    with TileContext(nc) as tc:
        with (
            tc.tile_pool(name="const", bufs=1) as cpool,
            tc.tile_pool(name="blk", bufs=1) as bpool,
            tc.tile_pool(name="work", bufs=6) as wpool,
            tc.tile_pool(name="outp", bufs=5) as opool,
            tc.tile_pool(name="psP", bufs=5, space="PSUM") as psP,
            tc.tile_pool(name="psR", bufs=3, space="PSUM") as psR,
        ):
            # warmup: PE p-state ramp on a zero tile
            warm = cpool.tile([128, 128], F16, tag="warm")
            nc.gpsimd.memset(warm[:, :], 0.0)
            warm_ps = psP.tile([128, NDEN], F32, tag="P", name="warmps")
            for wi in range(NWARM):
                nc.tensor.matmul(warm_ps[:, :128], warm[:, :], warm[:, :],
                                 start=True, stop=True)

            boot = cpool.tile([128, bootw], F16, tag="boot")
            nc.sync.dma_start(out=boot[:, :], in_=boot_d[:, :])
            gm = cpool.tile([128, GMW], F8, tag="gm")
            nc.sync.dma_start(out=gm[:, :], in_=gm_d[:, :])

            blk = [None] * NPB
            off_x = 0
            for pb in range(1, NPB):
                w = NG * BLK_ROWS[pb] * PW
                blk[pb] = bpool.tile([KC, w], F16, tag=f"blk{pb}",
                                     name=f"blk{pb}")
                src = AP(tensor=xrep_d, offset=off_x, ap=[[xrepw, KC], [1, w]])
                dst = AP(tensor=blk[pb].tensor, offset=blk[pb].offset,
                         ap=[[w, KC], [1, w]])
                nc.sync.dma_start(out=dst, in_=src)
                off_x += w

            def stage1(pb):
                nr = BLK_ROWS[pb]
                n = nr * W
                gw = nr * PW
                P = [psP.tile([128, NDEN], F32, tag="P", name=f"P{pb}_{m}")
                     for m in range(2)]
                # last two blocks run m-outer so P0's chain (and its copy)
                # completes 4 matmuls earlier, hiding copy latency in the
                # tail; earlier blocks stay m-inner
                gm_order = [(g, m) for m in range(2) for g in range(NG)]
                for g, m in gm_order:
                    if pb == 0:
                        t = boot
                        off, pitch = boot.offset + WCW + 64 + g * gw, bootw
                    else:
                        t = blk[pb]
                        off, pitch = t.offset + g * gw, NG * gw
                    rhs = AP(tensor=t.tensor, offset=off,
                             ap=[[pitch, KC], [PW, nr], [1, W]])
                    lt = boot[0:KC, (g * 2 + m) * 128:(g * 2 + m + 1) * 128]
                    nc.tensor.matmul(P[m][:, :n], lt, rhs,
                                     start=(g == 0), stop=(g == NG - 1))
                if pb >= NPB - F16_LAST:
                    # light tail path: fp16 A (ints <= 2048 exact), 2 copies
                    A = wpool.tile([128, 2 * NDEN], F16, tag="A16",
                                   name=f"A{pb}")
                    nc.scalar.copy(A[:, 0:n], P[0][:, :n])
                    nc.vector.tensor_copy(A[:, NDEN:NDEN + n], P[1][:, :n])
                    return A
                # A: fp8 tile [Ah0|Ah1|Al0|Al1]. P values are channel-summed
                # ints (|P| <~ 150): Dekker-split P = Ah + Al, both exact in
                # e4m3 (RNE residual of an int <= 256 is an int <= 8).
                A = wpool.tile([128, 4 * NDEN], F8, tag="A", name=f"A{pb}")
                nc.scalar.copy(A[:, 0:n], P[0][:, :n])
                nc.vector.scalar_tensor_tensor(
                    A[:, 2 * NDEN:2 * NDEN + n], P[0][:, :n], 0.0, A[:, 0:n],
                    mybir.AluOpType.add, mybir.AluOpType.subtract)
                nc.scalar.copy(A[:, NDEN:NDEN + n], P[1][:, :n])
                nc.vector.scalar_tensor_tensor(
                    A[:, 3 * NDEN:3 * NDEN + n], P[1][:, :n], 0.0,
                    A[:, NDEN:NDEN + n],
                    mybir.AluOpType.add, mybir.AluOpType.subtract)
                return A

            osb = None
            osb_off = 0
            osb_pb0 = 0
            pair_start = (list(PAIR_START) if PAIR_START is not None else
                          [True] + [(pb % 2) == (NPB % 2)
                                    for pb in range(1, NPB)])

            def stage2(pb, A):
                nonlocal osb, osb_off, osb_pb0
                nr = BLK_ROWS[pb]
                n = nr * W
                R = psR.tile([32, NDEN], F32, tag="R", name=f"R{pb}")
                if pb >= NPB - F16_LAST:
                    # fp16 A: two plain matmuls with the boot-resident gmat
                    nc.tensor.matmul(R[:, :n], boot[:, WCW:WCW + 32],
                                     A[:, 0:n], start=True, stop=False)
                    nc.tensor.matmul(R[:, :n], boot[:, WCW + 32:WCW + 64],
                                     A[:, NDEN:NDEN + n],
                                     start=False, stop=True)
                elif pb >= NPB - SPLIT_S2_LAST:
                    # four plain fp8 matmuls: each needs only one A quarter
                    for lo, ao in ((0, 0), (0, 2 * NDEN), (64, NDEN),
                                   (64, 3 * NDEN)):
                        nc.tensor.matmul(R[:, :n], gm[:, lo:lo + 32],
                                         A[:, ao:ao + n],
                                         start=(ao == 0), stop=(lo == 64 and
                                                                ao >= 3 * NDEN))
                else:
                    # two DoubleRow matmuls; pair dim = (Ah, Al) per m-half,
                    # so the m0 matmul only needs the m0 copies
                    for m in range(2):
                        lt = AP(tensor=gm.tensor, offset=gm.offset + 64 * m,
                                ap=[[GMW, 128], [32, 2], [1, 32]])
                        rhs = AP(tensor=A.tensor, offset=A.offset + m * NDEN,
                                 ap=[[4 * NDEN, 128], [2 * NDEN, 2], [1, n]])
                        nc.tensor.matmul(R[:, :n], lt, rhs, start=(m == 0),
                                         stop=(m == 1),
                                         perf_mode=mybir.MatmulPerfMode.DoubleRow)
                if pair_start[pb]:
                    osb = opool.tile([32, 2 * NDEN], F16, tag="osb",
                                     name=f"osb{pb}")
                    osb_off = 0
                    osb_pb0 = pb
                out_ap = AP(tensor=osb.tensor, offset=osb.offset + osb_off,
                            ap=[[2 * NDEN, 32], [1, n]])
                if R_ON_DVE(pb):
                    nc.vector.tensor_copy(out_ap, R[:, :n])
                else:
                    nc.scalar.copy(out_ap, R[:, :n])
                osb_off += n
                if pb == NPB - 1 or pair_start[pb + 1]:
                    pb0 = osb_pb0
                    nrows = sum(BLK_ROWS[pb0:pb + 1])
                    dst = AP(tensor=out_d, offset=BLK_R0[pb0] * W,
                             ap=[[H * W, OUT], [1, nrows * W]])
                    src = AP(tensor=osb.tensor, offset=osb.offset,
                             ap=[[2 * NDEN, 32], [1, nrows * W]])
                    nc.sync.dma_start(out=dst, in_=src)

            def lag_of(q):
                if q >= NPB - F16_LAST:
                    return 1
                # last two Dekker blocks: shorter lag so their R copies
                # free psR banks before the tail stage-2s need them (the
                # scheduler reorders around waits, so emitting earlier
                # lets them run at dep-ready time)
                return 2 if q >= NPB - F16_LAST - 2 else S2_LAG

            As = []
            nxt = 0
            for pb in range(NPB):
                As.append(stage1(pb))
                while nxt < NPB and nxt <= pb - lag_of(nxt):
                    stage2(nxt, As[nxt])
                    nxt += 1
            while nxt < NPB:
                stage2(nxt, As[nxt])
                nxt += 1

    _split_multiwaits(nc)
    return nc


def kernel(x, weight, bias):
    x = np.asarray(x, np.float32)
    weight = np.asarray(weight, np.float32)
    bias = np.asarray(bias, np.float32)

    parts = _build_xrep(x)
    wc, gmat = _prep_weights(weight, bias)
    bootw = WCW + 64 + NG * BLK_ROWS[0] * PW

    if "nc" not in _cache:
        _cache["nc"] = _build_nc()
    nc = _cache["nc"]

    in_maps = []
    for b in range(B):
        boot = np.zeros((128, bootw), np.float16)
        boot[:, :WCW + 64] = wc
        boot[:KC, WCW + 64:] = parts[0][b]
        xr = np.concatenate([p[b] for p in parts[1:]], axis=1)
        in_maps.append({"boot": boot, "gmat": gmat, "xrep": xr})
    res = run_bass_kernel_spmd(nc, in_maps, core_ids=list(range(B)))
    out = np.stack([np.asarray(r["out"], np.float32) for r in res.results])
    return (out * SCALE).reshape(B, OUT, H, W).astype(np.float32)


# revision 6
# speedup vs baseline: 1.0104x; 1.0040x over previous
"""Trainium2 Bass kernel for nn_Demolition_splitweight_Conv2d.

Computation (per batch element b, one NeuronCore each):
    out[o, p] = (1/(127*Q)) * sum_k wvec[k] * sum_c round(Q*(conv3x3(x[c]; w[k,c,o]) + b[k,c,o]))
with Q = 12.5, wvec = [-128, 1, 2, 4, 8, 16, 32, 64].

Stage 1 (unchanged from the 25702ns baseline): fp16 single-term products +
fp32 magic-number rounding inside the TensorEngine accumulation; per input
channel 12 contraction rows [9 taps, bias, +M, -M] (M = 3072*4096), 2
channels per 32-row PE segment (HW-probed sequential-within-segment
accumulation), 8 channels/pass, 4 chained passes x 2 out-halves per pixel
block -> P[m][(k,ol), px] = channel-summed rounded ints.

Stage 2 (new): bit recombination as fp8e4 DoubleRow matmuls at 0.5
cycles/col -- half the fp16 cost. P holds ints up to ~+-150, beyond e4m3's
exact-int range (16), so each P half is Dekker-split on the copy engines:
Ah = fp8(P) (Act copy, RNE), Al = P - Ah (DVE scalar_tensor_tensor); both
are exact in e4m3 (the RNE residual of an int <= 256 is an int <= 8). One
DoubleRow matmul per out-half with pair dim = (Ah, Al) and gmat pair slots
duplicated reconstructs sum_k WVEC[k]*P exactly (gmat entries are +-pow2,
e4m3-exact). R = un-scaled integer results; the host applies 1/(127*Q).
HW-verified: rel err 0.00928, identical to the baseline.

The last F16_LAST=3 (small) blocks instead use fp16 A + two plain matmuls:
at the tail the Dekker chain (P -> Ah -> Al, 5 PSUM-reads/block on 2
engines) cannot hide behind the tiny stage-1 blocks, and plain fp16 keeps
the final copy->DMA chain short. Stage-2 for Dekker blocks is emitted
S2_LAG=3 blocks late so the PE reaches it well after the copy chain lands
(lag 1 for the light fp16 tail to spread the osb DMAs).

Sim-swept schedule/pools (TimelineSim == the graded metric): blocks ramp
3,3,4,5,6 then 8s with a 5,4,2 tail; psP=5/psR=3 PSUM banks; outp=5
avoids the osb-pool back-pressure (osb tiles are freed only by DMA
completion +900ns sem prop). 30 dep-light warmup matmuls ramp the PE
p-state (2.4GHz needs 3us of continuous busy) under the boot DMA; real
work starts at the boot-DMA semaphore (~4.6us: preamble + HWDGE gen 625 +
DGE delay 650 + transfer 1337 + sem prop 900 -- feed-bound, a chunked boot
was tried and starves block 1). Measured: 24431 ns (TimelineSim), rel err
0.00928 on HW (baseline: 25702 ns / 0.00928; bf16 3-term kernel: 44011).
"""

import numpy as np
import ml_dtypes

import concourse.bass as bass
import concourse.mybir as mybir
from concourse.ap import AP
from concourse.tile import TileContext
from concourse.bass_utils import run_bass_kernel_spmd

# problem dims (hardcoded per the task contract)
B, C, OUT, H, W = 8, 32, 32, 64, 64
KBITS = 8
Q = 12.5
WVEC = np.array([-128, 1, 2, 4, 8, 16, 32, 64], np.float32)
SCALE = float(1.0 / (127.0 * Q))

PW = 66            # padded width  (1 + 64 + 1)
PH = 66            # padded height
PSZ = PH * PW      # 4356
NG = 4             # stage-1 passes: 4 x (4 segments x 2 channels)
KC = 120           # contraction rows per pass (last segment unpadded)
RPC = 12           # contraction rows per channel

BLK_ROWS = [3, 3, 4, 5, 6, 8, 8, 8, 8, 5, 4, 2]   # image rows per block
NPB = len(BLK_ROWS)
BLK_R0 = np.cumsum([0] + BLK_ROWS).tolist()        # first image row
NDEN = 512         # max dense moving dim: one PSUM bank exactly
PADW = 64 * PW + 134  # host pad buffer width

# engine policy: "Rpb on DVE?" / "A1 on DVE for last k blocks"
R_ON_DVE = lambda pb: ((pb % 2 == 0) and pb < NPB - 3) or pb >= NPB - 2
A1_ON_DVE_LAST = 1
SPLIT_S2_LAST = 0   # (subsumed by F16_LAST; kept for sweeps)
PAIR_START = None   # explicit osb pair starts (list of bool) or None
S2_LAG = 3          # emit stage2(pb) after stage1(pb + S2_LAG)
F16_LAST = 3        # last k blocks: fp16 A + 2 plain matmuls (light tail)
NWARM = 30          # PE p-state warmup matmuls (128 cols each)
WCW = 2 * NG * 128  # weight cols: 8 stage-1 lhsT blocks (gmat separate, fp8)
GMW = 128           # gmat pair cols: [gm0|gm0|gm1|gm1]
F16 = mybir.dt.float16
F32 = mybir.dt.float32
F8 = mybir.dt.float8e4
NP8 = ml_dtypes.float8_e4m3

_cache = {}


def _f16(a):
    return np.asarray(a, np.float32).astype(np.float16)


def _row_of(c, j):
    """Contraction row (pass g, partition w) for channel c, intra row j."""
    g, r = divmod(c, 8)
    s, t = divmod(r, 2)
    return g, s * 32 + t * RPC + j


def _prep_weights(weight, bias):
    """Stage-1 lhsT blocks [128, WCW] fp16 and gmat pairs [128, 64] fp8."""
    qw = _f16(Q * weight.astype(np.float32)).reshape(KBITS, C, 2, 16, 9)
    qb = _f16(Q * bias.astype(np.float32)).reshape(KBITS, C, 2, 16)

    Wt = np.zeros((NG, KC, 2, 128), np.float16)
    for c in range(C):
        for j in range(9):
            g, w = _row_of(c, j)
            Wt[g, w] = qw[:, c, :, :, j].transpose(1, 0, 2).reshape(2, 128)
        g, w = _row_of(c, 9)
        Wt[g, w] = qb[:, c].transpose(1, 0, 2).reshape(2, 128)
        g, w = _row_of(c, 10)
        Wt[g, w] = np.float16(3072.0)
        g, w = _row_of(c, 11)
        Wt[g, w] = np.float16(-3072.0)

    wc = np.zeros((128, WCW + 64), np.float16)
    for g in range(NG):
        for m in range(2):
            wc[:KC, (g * 2 + m) * 128:(g * 2 + m + 1) * 128] = Wt[g, :, m, :]

    j = np.arange(128)
    k_of, ol_of = j // 16, j % 16
    gm0 = np.zeros((128, 32), np.float32)
    gm1 = np.zeros((128, 32), np.float32)
    gm0[j, ol_of] = WVEC[k_of]        # m=0 half -> out rows 0..15
    gm1[j, 16 + ol_of] = WVEC[k_of]   # m=1 half -> out rows 16..31
    # fp16 gmat halves ride in the boot tile (pow2 values, fp16-exact)
    wc[:, WCW:WCW + 32] = gm0.astype(np.float16)
    wc[:, WCW + 32:WCW + 64] = gm1.astype(np.float16)
    # DoubleRow pair dim = (Ah, Al): both slots use the same gm half
    gmat = np.concatenate([gm0, gm0, gm1, gm1], axis=1)
    return wc, gmat.astype(NP8)


def _build_xrep(x):
    """Host REP: per-block [B, KC, NG*nr*PW] fp16 slices."""
    xh = _f16(x)
    xpad = np.zeros((B, C, PADW), np.float16)
    xpad[:, :, :PSZ].reshape(B, C, PH, PW)[:, :, 1:H + 1, 1:W + 1] = xh

    rep = np.zeros((B, NG, KC, H * PW), np.float16)
    for j in range(9):
        off = (j // 3) * PW + (j % 3)
        for c in range(C):
            g, w = _row_of(c, j)
            rep[:, g, w, :] = xpad[:, c, off:off + H * PW]
    for c in range(C):
        g, w = _row_of(c, 9)
        rep[:, g, w, :] = np.float16(1.0)
        for j in (10, 11):
            g, w = _row_of(c, j)
            rep[:, g, w, :] = np.float16(4096.0)
    parts = []
    for pb in range(NPB):
        r0, nr = BLK_R0[pb], BLK_ROWS[pb]
        sl = rep[:, :, :, r0 * PW:(r0 + nr) * PW]           # [B, NG, KC, w]
        parts.append(sl.transpose(0, 2, 1, 3).reshape(B, KC, NG * nr * PW))
    return parts


def _split_multiwaits(nc):
    """This container's walrus allows one sync-wait per instruction; move
    extras onto preceding same-engine NoOps."""
    for bb in nc.main_func.blocks:
        insts = bb.instructions
        i = 0
        while i < len(insts):
            ins = insts[i]
            si = getattr(ins, "sync_info", None)
            if si is not None and si.on_wait is not None and len(si.on_wait) > 1:
                waits = list(si.on_wait)
                nops = []
                for j, w in enumerate(waits[:-1]):
                    nop = mybir.InstNoOp(name=f"{ins.name}-wsplit{j}", ins=[], outs=[])
                    nop.engine = ins.engine
                    nop.sync_info = mybir.SyncInfo(on_wait=[w], on_update=[])
                    nops.append(nop)
                si.on_wait = [waits[-1]]
                ins.sync_info = si
                for j, nop in enumerate(nops):
                    insts.insert(i + j, nop)
                i += len(nops)
            i += 1


def _build_nc():
    bootw = WCW + 64 + NG * BLK_ROWS[0] * PW  # boot: weights+gm16+block 0
    xrepw = NG * (H - BLK_ROWS[0]) * PW      # xrep dram: blocks 1..NPB-1

    nc = bass.Bass()
    boot_d = nc.dram_tensor("boot", [128, bootw], F16, kind="ExternalInput")
    gm_d = nc.dram_tensor("gmat", [128, GMW], F8, kind="ExternalInput")
    xrep_d = nc.dram_tensor("xrep", [KC, xrepw], F16, kind="ExternalInput")
    out_d = nc.dram_tensor("out", [OUT, H * W], F16, kind="ExternalOutput")

    with TileContext(nc) as tc:
        with (
            tc.tile_pool(name="const", bufs=1) as cpool,
            tc.tile_pool(name="blk", bufs=1) as bpool,
            tc.tile_pool(name="work", bufs=6) as wpool,
            tc.tile_pool(name="outp", bufs=5) as opool,
            tc.tile_pool(name="psP", bufs=5, space="PSUM") as psP,
            tc.tile_pool(name="psR", bufs=3, space="PSUM") as psR,
        ):
            # warmup: PE p-state ramp on a zero tile
            warm = cpool.tile([128, 128], F16, tag="warm")
            nc.gpsimd.memset(warm[:, :], 0.0)
            warm_ps = psP.tile([128, NDEN], F32, tag="P", name="warmps")
            for wi in range(NWARM):
                nc.tensor.matmul(warm_ps[:, :128], warm[:, :], warm[:, :],
                                 start=True, stop=True)

            boot = cpool.tile([128, bootw], F16, tag="boot")
            nc.sync.dma_start(out=boot[:, :], in_=boot_d[:, :])
            gm = cpool.tile([128, GMW], F8, tag="gm")
            nc.sync.dma_start(out=gm[:, :], in_=gm_d[:, :])

            blk = [None] * NPB
            off_x = 0
            for pb in range(1, NPB):
                w = NG * BLK_ROWS[pb] * PW
                blk[pb] = bpool.tile([KC, w], F16, tag=f"blk{pb}",
                                     name=f"blk{pb}")
                src = AP(tensor=xrep_d, offset=off_x, ap=[[xrepw, KC], [1, w]])
                dst = AP(tensor=blk[pb].tensor, offset=blk[pb].offset,
                         ap=[[w, KC], [1, w]])
                nc.sync.dma_start(out=dst, in_=src)
                off_x += w

            def stage1(pb):
                nr = BLK_ROWS[pb]
                n = nr * W
                gw = nr * PW
                P = [psP.tile([128, NDEN], F32, tag="P", name=f"P{pb}_{m}")
                     for m in range(2)]
                # last two blocks run m-outer so P0's chain (and its copy)
                # completes 4 matmuls earlier, hiding copy latency in the
                # tail; earlier blocks stay m-inner
                gm_order = [(g, m) for m in range(2) for g in range(NG)]
                for g, m in gm_order:
                    if pb == 0:
                        t = boot
                        off, pitch = boot.offset + WCW + 64 + g * gw, bootw
                    else:
                        t = blk[pb]
                        off, pitch = t.offset + g * gw, NG * gw
                    rhs = AP(tensor=t.tensor, offset=off,
                             ap=[[pitch, KC], [PW, nr], [1, W]])
                    lt = boot[0:KC, (g * 2 + m) * 128:(g * 2 + m + 1) * 128]
                    nc.tensor.matmul(P[m][:, :n], lt, rhs,
                                     start=(g == 0), stop=(g == NG - 1))
                if pb >= NPB - F16_LAST:
                    # light tail path: fp16 A (ints <= 2048 exact), 2 copies
                    A = wpool.tile([128, 2 * NDEN], F16, tag="A16",
                                   name=f"A{pb}")
                    nc.scalar.copy(A[:, 0:n], P[0][:, :n])
                    nc.vector.tensor_copy(A[:, NDEN:NDEN + n], P[1][:, :n])
                    return A
                # A: fp8 tile [Ah0|Ah1|Al0|Al1]. P values are channel-summed
                # ints (|P| <~ 150): Dekker-split P = Ah + Al, both exact in
                # e4m3 (RNE residual of an int <= 256 is an int <= 8).
                A = wpool.tile([128, 4 * NDEN], F8, tag="A", name=f"A{pb}")
                nc.scalar.copy(A[:, 0:n], P[0][:, :n])
                nc.vector.scalar_tensor_tensor(
                    A[:, 2 * NDEN:2 * NDEN + n], P[0][:, :n], 0.0, A[:, 0:n],
                    mybir.AluOpType.add, mybir.AluOpType.subtract)
                nc.scalar.copy(A[:, NDEN:NDEN + n], P[1][:, :n])
                nc.vector.scalar_tensor_tensor(
                    A[:, 3 * NDEN:3 * NDEN + n], P[1][:, :n], 0.0,
                    A[:, NDEN:NDEN + n],
                    mybir.AluOpType.add, mybir.AluOpType.subtract)
                return A

            osb = None
            osb_off = 0
            osb_pb0 = 0
            pair_start = (list(PAIR_START) if PAIR_START is not None else
                          [True] + [(pb % 2) == (NPB % 2)
                                    for pb in range(1, NPB)])

            def stage2(pb, A):
                nonlocal osb, osb_off, osb_pb0
                nr = BLK_ROWS[pb]
                n = nr * W
                R = psR.tile([32, NDEN], F32, tag="R", name=f"R{pb}")
                if pb >= NPB - F16_LAST:
                    # fp16 A: two plain matmuls with the boot-resident gmat
                    nc.tensor.matmul(R[:, :n], boot[:, WCW:WCW + 32],
                                     A[:, 0:n], start=True, stop=False)
                    nc.tensor.matmul(R[:, :n], boot[:, WCW + 32:WCW + 64],
                                     A[:, NDEN:NDEN + n],
                                     start=False, stop=True)
                elif pb >= NPB - SPLIT_S2_LAST:
                    # four plain fp8 matmuls: each needs only one A quarter
                    for lo, ao in ((0, 0), (0, 2 * NDEN), (64, NDEN),
                                   (64, 3 * NDEN)):
                        nc.tensor.matmul(R[:, :n], gm[:, lo:lo + 32],
                                         A[:, ao:ao + n],
                                         start=(ao == 0), stop=(lo == 64 and
                                                                ao >= 3 * NDEN))
                else:
                    # two DoubleRow matmuls; pair dim = (Ah, Al) per m-half,
                    # so the m0 matmul only needs the m0 copies
                    for m in range(2):
                        lt = AP(tensor=gm.tensor, offset=gm.offset + 64 * m,
                                ap=[[GMW, 128], [32, 2], [1, 32]])
                        rhs = AP(tensor=A.tensor, offset=A.offset + m * NDEN,
                                 ap=[[4 * NDEN, 128], [2 * NDEN, 2], [1, n]])
                        nc.tensor.matmul(R[:, :n], lt, rhs, start=(m == 0),
                                         stop=(m == 1),
                                         perf_mode=mybir.MatmulPerfMode.DoubleRow)
                if pair_start[pb]:
                    osb = opool.tile([32, 2 * NDEN], F16, tag="osb",
                                     name=f"osb{pb}")
                    osb_off = 0
                    osb_pb0 = pb
                out_ap = AP(tensor=osb.tensor, offset=osb.offset + osb_off,
                            ap=[[2 * NDEN, 32], [1, n]])
                if R_ON_DVE(pb):
                    nc.vector.tensor_copy(out_ap, R[:, :n])
                else:
                    nc.scalar.copy(out_ap, R[:, :n])
                osb_off += n
                if pb == NPB - 1 or pair_start[pb + 1]:
                    pb0 = osb_pb0
                    nrows = sum(BLK_ROWS[pb0:pb + 1])
                    dst = AP(tensor=out_d, offset=BLK_R0[pb0] * W,
                             ap=[[H * W, OUT], [1, nrows * W]])
                    src = AP(tensor=osb.tensor, offset=osb.offset,
                             ap=[[2 * NDEN, 32], [1, nrows * W]])
                    nc.sync.dma_start(out=dst, in_=src)

            def lag_of(q):
                if q >= NPB - F16_LAST:
                    return 1
                # last two Dekker blocks: shorter lag so their R copies
                # free psR banks before the tail stage-2s need them (the
                # scheduler reorders around waits, so emitting earlier
                # lets them run at dep-ready time)
                return 2 if q >= NPB - F16_LAST - 2 else S2_LAG

            As = []
            nxt = 0
            for pb in range(NPB):
                As.append(stage1(pb))
                while nxt < NPB and nxt <= pb - lag_of(nxt):
                    stage2(nxt, As[nxt])
                    nxt += 1
            while nxt < NPB:
                stage2(nxt, As[nxt])
                nxt += 1

    _split_multiwaits(nc)
    return nc


def kernel(x, weight, bias):
    x = np.asarray(x, np.float32)
    weight = np.asarray(weight, np.float32)
    bias = np.asarray(bias, np.float32)

    parts = _build_xrep(x)
    wc, gmat = _prep_weights(weight, bias)
    bootw = WCW + 64 + NG * BLK_ROWS[0] * PW

    if "nc" not in _cache:
        _cache["nc"] = _build_nc()
    nc = _cache["nc"]

    in_maps = []
    for b in range(B):
        boot = np.zeros((128, bootw), np.float16)
        boot[:, :WCW + 64] = wc
        boot[:KC, WCW + 64:] = parts[0][b]
        xr = np.concatenate([p[b] for p in parts[1:]], axis=1)
        in_maps.append({"boot": boot, "gmat": gmat, "xrep": xr})
    res = run_bass_kernel_spmd(nc, in_maps, core_ids=list(range(B)))
    out = np.stack([np.asarray(r["out"], np.float32) for r in res.results])
    return (out * SCALE).reshape(B, OUT, H, W).astype(np.float32)


# revision 7
# speedup vs baseline: 1.0145x; 1.0041x over previous
"""Trainium2 Bass kernel for nn_Demolition_splitweight_Conv2d.

Computation (per batch element b, one NeuronCore each):
    out[o, p] = (1/(127*Q)) * sum_k wvec[k] * sum_c round(Q*(conv3x3(x[c]; w[k,c,o]) + b[k,c,o]))
with Q = 12.5, wvec = [-128, 1, 2, 4, 8, 16, 32, 64].

Stage 1 (unchanged from the 25702ns baseline): fp16 single-term products +
fp32 magic-number rounding inside the TensorEngine accumulation; per input
channel 12 contraction rows [9 taps, bias, +M, -M] (M = 3072*4096), 2
channels per 32-row PE segment (HW-probed sequential-within-segment
accumulation), 8 channels/pass, 4 chained passes x 2 out-halves per pixel
block -> P[m][(k,ol), px] = channel-summed rounded ints.

Stage 2 (new): bit recombination as fp8e4 DoubleRow matmuls at 0.5
cycles/col -- half the fp16 cost. P holds ints up to ~+-150, beyond e4m3's
exact-int range (16), so each P half is Dekker-split on the copy engines:
Ah = fp8(P) (Act copy, RNE), Al = P - Ah (DVE scalar_tensor_tensor); both
are exact in e4m3 (the RNE residual of an int <= 256 is an int <= 8). One
DoubleRow matmul per out-half with pair dim = (Ah, Al) and gmat pair slots
duplicated reconstructs sum_k WVEC[k]*P exactly (gmat entries are +-pow2,
e4m3-exact). R = un-scaled integer results; the host applies 1/(127*Q).
HW-verified: rel err 0.00928, identical to the baseline.

The last F16_LAST=3 (small) blocks instead use fp16 A + two plain matmuls:
at the tail the Dekker chain (P -> Ah -> Al, 5 PSUM-reads/block on 2
engines) cannot hide behind the tiny stage-1 blocks, and plain fp16 keeps
the final copy->DMA chain short. Stage-2 for Dekker blocks is emitted
S2_LAG=3 blocks late so the PE reaches it well after the copy chain lands
(lag 1 for the light fp16 tail to spread the osb DMAs).

Sim-swept schedule/pools (TimelineSim == the graded metric): blocks ramp
3,3,4,5,6 then 8s with a 5,4,2 tail; psP=5/psR=3 PSUM banks; outp=5
avoids the osb-pool back-pressure (osb tiles are freed only by DMA
completion +900ns sem prop). 30 dep-light warmup matmuls ramp the PE
p-state (2.4GHz needs 3us of continuous busy) under the boot DMA; real
work starts at the boot-DMA semaphore (~4.5us: preamble + HWDGE gen 625 +
DGE delay 650 + transfer + sem prop 900 -- feed-bound; a chunked boot
starves block 1, and partition-skipping uploads of the 24 zero segment-pad
rows are inexpressible as one DMA). The boot DRAM tensor carries only the
KC=120 live partitions, and the fp16 tail gmat is derived on-device from
the fp8 copy (one early DVE op) instead of riding the boot payload. Measured: 24332 ns (TimelineSim), rel err
0.00928 on HW (baseline: 25702 ns / 0.00928; bf16 3-term kernel: 44011).
"""

import numpy as np
import ml_dtypes

import concourse.bass as bass
import concourse.mybir as mybir
from concourse.ap import AP
from concourse.tile import TileContext
from concourse.bass_utils import run_bass_kernel_spmd

# problem dims (hardcoded per the task contract)
B, C, OUT, H, W = 8, 32, 32, 64, 64
KBITS = 8
Q = 12.5
WVEC = np.array([-128, 1, 2, 4, 8, 16, 32, 64], np.float32)
SCALE = float(1.0 / (127.0 * Q))

PW = 66            # padded width  (1 + 64 + 1)
PH = 66            # padded height
PSZ = PH * PW      # 4356
NG = 4             # stage-1 passes: 4 x (4 segments x 2 channels)
KC = 120           # contraction rows per pass (last segment unpadded)
RPC = 12           # contraction rows per channel

BLK_ROWS = [3, 3, 4, 5, 6, 8, 8, 8, 8, 5, 4, 2]   # image rows per block
NPB = len(BLK_ROWS)
BLK_R0 = np.cumsum([0] + BLK_ROWS).tolist()        # first image row
NDEN = 512         # max dense moving dim: one PSUM bank exactly
PADW = 64 * PW + 134  # host pad buffer width

# engine policy: "Rpb on DVE?" / "A1 on DVE for last k blocks"
R_ON_DVE = lambda pb: ((pb % 2 == 0) and pb < NPB - 3) or pb >= NPB - 2
A1_ON_DVE_LAST = 1
SPLIT_S2_LAST = 0   # (subsumed by F16_LAST; kept for sweeps)
PAIR_START = None   # explicit osb pair starts (list of bool) or None
S2_LAG = 3          # emit stage2(pb) after stage1(pb + S2_LAG)
F16_LAST = 3        # last k blocks: fp16 A + 2 plain matmuls (light tail)
NWARM = 29          # PE p-state warmup matmuls (128 cols each)
WCW = 2 * NG * 128  # weight cols: 8 stage-1 lhsT blocks (gmat separate, fp8)
GMW = 128           # gmat pair cols: [gm0|gm0|gm1|gm1]
F16 = mybir.dt.float16
F32 = mybir.dt.float32
F8 = mybir.dt.float8e4
NP8 = ml_dtypes.float8_e4m3

_cache = {}


def _f16(a):
    return np.asarray(a, np.float32).astype(np.float16)


def _row_of(c, j):
    """Contraction row (pass g, partition w) for channel c, intra row j."""
    g, r = divmod(c, 8)
    s, t = divmod(r, 2)
    return g, s * 32 + t * RPC + j


def _prep_weights(weight, bias):
    """Stage-1 lhsT blocks [KC, WCW] fp16 and gmat pairs [128, GMW] fp8."""
    qw = _f16(Q * weight.astype(np.float32)).reshape(KBITS, C, 2, 16, 9)
    qb = _f16(Q * bias.astype(np.float32)).reshape(KBITS, C, 2, 16)

    Wt = np.zeros((NG, KC, 2, 128), np.float16)
    for c in range(C):
        for j in range(9):
            g, w = _row_of(c, j)
            Wt[g, w] = qw[:, c, :, :, j].transpose(1, 0, 2).reshape(2, 128)
        g, w = _row_of(c, 9)
        Wt[g, w] = qb[:, c].transpose(1, 0, 2).reshape(2, 128)
        g, w = _row_of(c, 10)
        Wt[g, w] = np.float16(3072.0)
        g, w = _row_of(c, 11)
        Wt[g, w] = np.float16(-3072.0)

    wc = np.zeros((KC, WCW), np.float16)
    for g in range(NG):
        for m in range(2):
            wc[:KC, (g * 2 + m) * 128:(g * 2 + m + 1) * 128] = Wt[g, :, m, :]

    j = np.arange(128)
    k_of, ol_of = j // 16, j % 16
    gm0 = np.zeros((128, 32), np.float32)
    gm1 = np.zeros((128, 32), np.float32)
    gm0[j, ol_of] = WVEC[k_of]        # m=0 half -> out rows 0..15
    gm1[j, 16 + ol_of] = WVEC[k_of]   # m=1 half -> out rows 16..31
    # DoubleRow pair dim = (Ah, Al): both slots use the same gm half
    gmat = np.concatenate([gm0, gm0, gm1, gm1], axis=1)
    return wc, gmat.astype(NP8)


def _build_xrep(x):
    """Host REP: per-block [B, KC, NG*nr*PW] fp16 slices."""
    xh = _f16(x)
    xpad = np.zeros((B, C, PADW), np.float16)
    xpad[:, :, :PSZ].reshape(B, C, PH, PW)[:, :, 1:H + 1, 1:W + 1] = xh

    rep = np.zeros((B, NG, KC, H * PW), np.float16)
    for j in range(9):
        off = (j // 3) * PW + (j % 3)
        for c in range(C):
            g, w = _row_of(c, j)
            rep[:, g, w, :] = xpad[:, c, off:off + H * PW]
    for c in range(C):
        g, w = _row_of(c, 9)
        rep[:, g, w, :] = np.float16(1.0)
        for j in (10, 11):
            g, w = _row_of(c, j)
            rep[:, g, w, :] = np.float16(4096.0)
    parts = []
    for pb in range(NPB):
        r0, nr = BLK_R0[pb], BLK_ROWS[pb]
        sl = rep[:, :, :, r0 * PW:(r0 + nr) * PW]           # [B, NG, KC, w]
        parts.append(sl.transpose(0, 2, 1, 3).reshape(B, KC, NG * nr * PW))
    return parts


def _split_multiwaits(nc):
    """This container's walrus allows one sync-wait per instruction; move
    extras onto preceding same-engine NoOps."""
    for bb in nc.main_func.blocks:
        insts = bb.instructions
        i = 0
        while i < len(insts):
            ins = insts[i]
            si = getattr(ins, "sync_info", None)
            if si is not None and si.on_wait is not None and len(si.on_wait) > 1:
                waits = list(si.on_wait)
                nops = []
                for j, w in enumerate(waits[:-1]):
                    nop = mybir.InstNoOp(name=f"{ins.name}-wsplit{j}", ins=[], outs=[])
                    nop.engine = ins.engine
                    nop.sync_info = mybir.SyncInfo(on_wait=[w], on_update=[])
                    nops.append(nop)
                si.on_wait = [waits[-1]]
                ins.sync_info = si
                for j, nop in enumerate(nops):
                    insts.insert(i + j, nop)
                i += len(nops)
            i += 1


def _build_nc():
    # boot carries only the KC=120 live partitions: -6% transfer time on
    # the critical start path (real work is gated by the boot-DMA sem)
    bootw = WCW + NG * BLK_ROWS[0] * PW      # boot: weights + block 0
    xrepw = NG * (H - BLK_ROWS[0]) * PW      # xrep dram: blocks 1..NPB-1

    nc = bass.Bass()
    boot_d = nc.dram_tensor("boot", [KC, bootw], F16, kind="ExternalInput")
    gm_d = nc.dram_tensor("gmat", [128, GMW], F8, kind="ExternalInput")
    xrep_d = nc.dram_tensor("xrep", [KC, xrepw], F16, kind="ExternalInput")
    out_d = nc.dram_tensor("out", [OUT, H * W], F16, kind="ExternalOutput")

    with TileContext(nc) as tc:
        with (
            tc.tile_pool(name="const", bufs=1) as cpool,
            tc.tile_pool(name="blk", bufs=1) as bpool,
            tc.tile_pool(name="work", bufs=6) as wpool,
            tc.tile_pool(name="outp", bufs=5) as opool,
            tc.tile_pool(name="psP", bufs=5, space="PSUM") as psP,
            tc.tile_pool(name="psR", bufs=3, space="PSUM") as psR,
        ):
            # warmup: PE p-state ramp on a zero tile
            warm = cpool.tile([128, 128], F16, tag="warm")
            nc.gpsimd.memset(warm[:, :], 0.0)
            warm_ps = psP.tile([128, NDEN], F32, tag="P", name="warmps")
            for wi in range(NWARM):
                nc.tensor.matmul(warm_ps[:, :128], warm[:, :], warm[:, :],
                                 start=True, stop=True)

            boot = cpool.tile([KC, bootw], F16, tag="boot")
            nc.sync.dma_start(out=boot[:, :], in_=boot_d[:, :])
            gm = cpool.tile([128, GMW], F8, tag="gm")
            nc.sync.dma_start(out=gm[:, :], in_=gm_d[:, :])
            # fp16 gmat for the tail path: derived on-device from the fp8
            # copy (pow2 values, both dtypes exact) instead of riding boot
            gm16 = cpool.tile([128, 64], F16, tag="gm16")
            nc.vector.tensor_copy(
                AP(tensor=gm16.tensor, offset=gm16.offset,
                   ap=[[64, 128], [32, 2], [1, 32]]),
                AP(tensor=gm.tensor, offset=gm.offset,
                   ap=[[GMW, 128], [64, 2], [1, 32]]))

            blk = [None] * NPB
            off_x = 0
            for pb in range(1, NPB):
                w = NG * BLK_ROWS[pb] * PW
                blk[pb] = bpool.tile([KC, w], F16, tag=f"blk{pb}",
                                     name=f"blk{pb}")
                src = AP(tensor=xrep_d, offset=off_x, ap=[[xrepw, KC], [1, w]])
                dst = AP(tensor=blk[pb].tensor, offset=blk[pb].offset,
                         ap=[[w, KC], [1, w]])
                nc.sync.dma_start(out=dst, in_=src)
                off_x += w

            def stage1(pb):
                nr = BLK_ROWS[pb]
                n = nr * W
                gw = nr * PW
                P = [psP.tile([128, NDEN], F32, tag="P", name=f"P{pb}_{m}")
                     for m in range(2)]
                # last two blocks run m-outer so P0's chain (and its copy)
                # completes 4 matmuls earlier, hiding copy latency in the
                # tail; earlier blocks stay m-inner
                gm_order = [(g, m) for m in range(2) for g in range(NG)]
                for g, m in gm_order:
                    if pb == 0:
                        t = boot
                        off, pitch = boot.offset + WCW + g * gw, bootw
                    else:
                        t = blk[pb]
                        off, pitch = t.offset + g * gw, NG * gw
                    rhs = AP(tensor=t.tensor, offset=off,
                             ap=[[pitch, KC], [PW, nr], [1, W]])
                    lt = boot[0:KC, (g * 2 + m) * 128:(g * 2 + m + 1) * 128]
                    nc.tensor.matmul(P[m][:, :n], lt, rhs,
                                     start=(g == 0), stop=(g == NG - 1))
                if pb >= NPB - F16_LAST:
                    # light tail path: fp16 A (ints <= 2048 exact), 2 copies
                    A = wpool.tile([128, 2 * NDEN], F16, tag="A16",
                                   name=f"A{pb}")
                    nc.scalar.copy(A[:, 0:n], P[0][:, :n])
                    nc.vector.tensor_copy(A[:, NDEN:NDEN + n], P[1][:, :n])
                    return A
                # A: fp8 tile [Ah0|Ah1|Al0|Al1]. P values are channel-summed
                # ints (|P| <~ 150): Dekker-split P = Ah + Al, both exact in
                # e4m3 (RNE residual of an int <= 256 is an int <= 8).
                A = wpool.tile([128, 4 * NDEN], F8, tag="A", name=f"A{pb}")
                nc.scalar.copy(A[:, 0:n], P[0][:, :n])
                nc.vector.scalar_tensor_tensor(
                    A[:, 2 * NDEN:2 * NDEN + n], P[0][:, :n], 0.0, A[:, 0:n],
                    mybir.AluOpType.add, mybir.AluOpType.subtract)
                nc.scalar.copy(A[:, NDEN:NDEN + n], P[1][:, :n])
                nc.vector.scalar_tensor_tensor(
                    A[:, 3 * NDEN:3 * NDEN + n], P[1][:, :n], 0.0,
                    A[:, NDEN:NDEN + n],
                    mybir.AluOpType.add, mybir.AluOpType.subtract)
                return A

            osb = None
            osb_off = 0
            osb_pb0 = 0
            pair_start = (list(PAIR_START) if PAIR_START is not None else
                          [True] + [(pb % 2) == (NPB % 2)
                                    for pb in range(1, NPB)])

            def stage2(pb, A):
                nonlocal osb, osb_off, osb_pb0
                nr = BLK_ROWS[pb]
                n = nr * W
                R = psR.tile([32, NDEN], F32, tag="R", name=f"R{pb}")
                if pb >= NPB - F16_LAST:
                    # fp16 A: two plain matmuls with the derived fp16 gmat
                    nc.tensor.matmul(R[:, :n], gm16[:, 0:32],
                                     A[:, 0:n], start=True, stop=False)
                    nc.tensor.matmul(R[:, :n], gm16[:, 32:64],
                                     A[:, NDEN:NDEN + n],
                                     start=False, stop=True)
                elif pb >= NPB - SPLIT_S2_LAST:
                    # four plain fp8 matmuls: each needs only one A quarter
                    for lo, ao in ((0, 0), (0, 2 * NDEN), (64, NDEN),
                                   (64, 3 * NDEN)):
                        nc.tensor.matmul(R[:, :n], gm[:, lo:lo + 32],
                                         A[:, ao:ao + n],
                                         start=(ao == 0), stop=(lo == 64 and
                                                                ao >= 3 * NDEN))
                else:
                    # two DoubleRow matmuls; pair dim = (Ah, Al) per m-half,
                    # so the m0 matmul only needs the m0 copies
                    for m in range(2):
                        lt = AP(tensor=gm.tensor, offset=gm.offset + 64 * m,
                                ap=[[GMW, 128], [32, 2], [1, 32]])
                        rhs = AP(tensor=A.tensor, offset=A.offset + m * NDEN,
                                 ap=[[4 * NDEN, 128], [2 * NDEN, 2], [1, n]])
                        nc.tensor.matmul(R[:, :n], lt, rhs, start=(m == 0),
                                         stop=(m == 1),
                                         perf_mode=mybir.MatmulPerfMode.DoubleRow)
                if pair_start[pb]:
                    osb = opool.tile([32, 2 * NDEN], F16, tag="osb",
                                     name=f"osb{pb}")
                    osb_off = 0
                    osb_pb0 = pb
                out_ap = AP(tensor=osb.tensor, offset=osb.offset + osb_off,
                            ap=[[2 * NDEN, 32], [1, n]])
                if R_ON_DVE(pb):
                    nc.vector.tensor_copy(out_ap, R[:, :n])
                else:
                    nc.scalar.copy(out_ap, R[:, :n])
                osb_off += n
                if pb == NPB - 1 or pair_start[pb + 1]:
                    pb0 = osb_pb0
                    nrows = sum(BLK_ROWS[pb0:pb + 1])
                    dst = AP(tensor=out_d, offset=BLK_R0[pb0] * W,
                             ap=[[H * W, OUT], [1, nrows * W]])
                    src = AP(tensor=osb.tensor, offset=osb.offset,
                             ap=[[2 * NDEN, 32], [1, nrows * W]])
                    nc.sync.dma_start(out=dst, in_=src)

            def lag_of(q):
                if q >= NPB - F16_LAST:
                    return 1
                # last two Dekker blocks: shorter lag so their R copies
                # free psR banks before the tail stage-2s need them (the
                # scheduler reorders around waits, so emitting earlier
                # lets them run at dep-ready time)
                return 2 if q >= NPB - F16_LAST - 2 else S2_LAG

            As = []
            nxt = 0
            for pb in range(NPB):
                As.append(stage1(pb))
                while nxt < NPB and nxt <= pb - lag_of(nxt):
                    stage2(nxt, As[nxt])
                    nxt += 1
            while nxt < NPB:
                stage2(nxt, As[nxt])
                nxt += 1

    _split_multiwaits(nc)
    return nc


def kernel(x, weight, bias):
    x = np.asarray(x, np.float32)
    weight = np.asarray(weight, np.float32)
    bias = np.asarray(bias, np.float32)

    parts = _build_xrep(x)
    wc, gmat = _prep_weights(weight, bias)
    bootw = WCW + NG * BLK_ROWS[0] * PW

    if "nc" not in _cache:
        _cache["nc"] = _build_nc()
    nc = _cache["nc"]

    in_maps = []
    for b in range(B):
        boot = np.zeros((KC, bootw), np.float16)
        boot[:, :WCW] = wc
        boot[:, WCW:] = parts[0][b]
        xr = np.concatenate([p[b] for p in parts[1:]], axis=1)
        in_maps.append({"boot": boot, "gmat": gmat, "xrep": xr})
    res = run_bass_kernel_spmd(nc, in_maps, core_ids=list(range(B)))
    out = np.stack([np.asarray(r["out"], np.float32) for r in res.results])
    return (out * SCALE).reshape(B, OUT, H, W).astype(np.float32)


# revision 9
# speedup vs baseline: 1.0163x; 1.0018x over previous
"""Trainium2 Bass kernel for nn_Demolition_splitweight_Conv2d.

Computation (per batch element b, one NeuronCore each):
    out[o, p] = (1/(127*Q)) * sum_k wvec[k] * sum_c round(Q*(conv3x3(x[c]; w[k,c,o]) + b[k,c,o]))
with Q = 12.5, wvec = [-128, 1, 2, 4, 8, 16, 32, 64].

Stage 1 (unchanged from the 25702ns baseline): fp16 single-term products +
fp32 magic-number rounding inside the TensorEngine accumulation; per input
channel 12 contraction rows [9 taps, bias, +M, -M] (M = 3072*4096), 2
channels per 32-row PE segment (HW-probed sequential-within-segment
accumulation), 8 channels/pass, 4 chained passes x 2 out-halves per pixel
block -> P[m][(k,ol), px] = channel-summed rounded ints.

Stage 2 (new): bit recombination as fp8e4 DoubleRow matmuls at 0.5
cycles/col -- half the fp16 cost. P holds ints up to ~+-150, beyond e4m3's
exact-int range (16), so each P half is Dekker-split on the copy engines:
Ah = fp8(P) (Act copy, RNE), Al = P - Ah (DVE scalar_tensor_tensor); both
are exact in e4m3 (the RNE residual of an int <= 256 is an int <= 8). One
DoubleRow matmul per out-half with pair dim = (Ah, Al) and gmat pair slots
duplicated reconstructs sum_k WVEC[k]*P exactly (gmat entries are +-pow2,
e4m3-exact). R = un-scaled integer results; the host applies 1/(127*Q).
HW-verified: rel err 0.00928, identical to the baseline.

The last F16_LAST=3 (small) blocks instead use fp16 A + two plain matmuls:
at the tail the Dekker chain (P -> Ah -> Al, 5 PSUM-reads/block on 2
engines) cannot hide behind the tiny stage-1 blocks, and plain fp16 keeps
the final copy->DMA chain short. Stage-2 for Dekker blocks is emitted
S2_LAG=3 blocks late so the PE reaches it well after the copy chain lands
(lag 1 for the light fp16 tail to spread the osb DMAs).

Sim-swept schedule/pools (TimelineSim == the graded metric): blocks ramp
3,3,4,5,6 then 8s with a 5,4,2 tail; psP=5/psR=3 PSUM banks; outp=5
avoids the osb-pool back-pressure (osb tiles are freed only by DMA
completion +900ns sem prop). 29 dep-light warmup matmuls ramp the PE
p-state (2.4GHz needs 3us of continuous busy) under the boot DMA; real
work starts at the boot-DMA semaphore (~4.5us: preamble + HWDGE gen 625 +
DGE delay 650 + transfer + sem prop 900 -- feed-bound; a chunked boot
starves block 1, and partition-skipping uploads of the 24 zero segment-pad
rows are inexpressible as one DMA). The boot DRAM tensor carries only the
KC=120 live partitions, and the fp16 tail gmat is derived on-device from
the fp8 copy (one early DVE op) instead of riding the boot payload. Measured: 24289 ns (TimelineSim), rel err
0.00928 on HW (baseline: 25702 ns / 0.00928; bf16 3-term kernel: 44011).
"""

import numpy as np
import ml_dtypes

import concourse.bass as bass
import concourse.mybir as mybir
from concourse.ap import AP
from concourse.tile import TileContext
from concourse.bass_utils import run_bass_kernel_spmd

# problem dims (hardcoded per the task contract)
B, C, OUT, H, W = 8, 32, 32, 64, 64
KBITS = 8
Q = 12.5
WVEC = np.array([-128, 1, 2, 4, 8, 16, 32, 64], np.float32)
SCALE = float(1.0 / (127.0 * Q))

PW = 66            # padded width  (1 + 64 + 1)
PH = 66            # padded height
PSZ = PH * PW      # 4356
NG = 4             # stage-1 passes: 4 x (4 segments x 2 channels)
KC = 120           # contraction rows per pass (last segment unpadded)
RPC = 12           # contraction rows per channel

BLK_ROWS = [3, 3, 4, 5, 6, 8, 8, 8, 8, 5, 4, 2]   # image rows per block
NPB = len(BLK_ROWS)
BLK_R0 = np.cumsum([0] + BLK_ROWS).tolist()        # first image row
NDEN = 512         # max dense moving dim: one PSUM bank exactly
PADW = 64 * PW + 134  # host pad buffer width

# engine policy: "Rpb on DVE?" / "A1 on DVE for last k blocks"
R_ON_DVE = lambda pb: ((pb % 2 == 0) and pb < NPB - 3) or pb >= NPB - 2
A1_ON_DVE_LAST = 1
SPLIT_S2_LAST = 0   # (subsumed by F16_LAST; kept for sweeps)
PAIR_START = None   # explicit osb pair starts (list of bool) or None
S2_LAG = 3          # emit stage2(pb) after stage1(pb + S2_LAG)
F16_LAST = 3        # last k blocks: fp16 A + 2 plain matmuls (light tail)
NWARM = 29          # PE p-state warmup matmuls (128 cols each)
WCW = 2 * NG * 128  # weight cols: 8 stage-1 lhsT blocks (gmat separate, fp8)
GMW = 128           # gmat pair cols: [gm0|gm0|gm1|gm1]
F16 = mybir.dt.float16
F32 = mybir.dt.float32
F8 = mybir.dt.float8e4
NP8 = ml_dtypes.float8_e4m3

_cache = {}


def _f16(a):
    return np.asarray(a, np.float32).astype(np.float16)


def _row_of(c, j):
    """Contraction row (pass g, partition w) for channel c, intra row j."""
    g, r = divmod(c, 8)
    s, t = divmod(r, 2)
    return g, s * 32 + t * RPC + j


def _prep_weights(weight, bias):
    """Stage-1 lhsT blocks [KC, WCW] fp16 and gmat pairs [128, GMW] fp8."""
    qw = _f16(Q * weight.astype(np.float32)).reshape(KBITS, C, 2, 16, 9)
    qb = _f16(Q * bias.astype(np.float32)).reshape(KBITS, C, 2, 16)

    Wt = np.zeros((NG, KC, 2, 128), np.float16)
    for c in range(C):
        for j in range(9):
            g, w = _row_of(c, j)
            Wt[g, w] = qw[:, c, :, :, j].transpose(1, 0, 2).reshape(2, 128)
        g, w = _row_of(c, 9)
        Wt[g, w] = qb[:, c].transpose(1, 0, 2).reshape(2, 128)
        g, w = _row_of(c, 10)
        Wt[g, w] = np.float16(3072.0)
        g, w = _row_of(c, 11)
        Wt[g, w] = np.float16(-3072.0)

    wc = np.zeros((KC, WCW), np.float16)
    for g in range(NG):
        for m in range(2):
            wc[:KC, (g * 2 + m) * 128:(g * 2 + m + 1) * 128] = Wt[g, :, m, :]

    j = np.arange(128)
    k_of, ol_of = j // 16, j % 16
    gm0 = np.zeros((128, 32), np.float32)
    gm1 = np.zeros((128, 32), np.float32)
    gm0[j, ol_of] = WVEC[k_of]        # m=0 half -> out rows 0..15
    gm1[j, 16 + ol_of] = WVEC[k_of]   # m=1 half -> out rows 16..31
    # DoubleRow pair dim = (Ah, Al): both slots use the same gm half
    gmat = np.concatenate([gm0, gm0, gm1, gm1], axis=1)
    return wc, gmat.astype(NP8)


def _build_xrep(x):
    """Host REP: per-block [B, KC, NG*nr*PW] fp16 slices."""
    xh = _f16(x)
    xpad = np.zeros((B, C, PADW), np.float16)
    xpad[:, :, :PSZ].reshape(B, C, PH, PW)[:, :, 1:H + 1, 1:W + 1] = xh

    rep = np.zeros((B, NG, KC, H * PW), np.float16)
    for j in range(9):
        off = (j // 3) * PW + (j % 3)
        for c in range(C):
            g, w = _row_of(c, j)
            rep[:, g, w, :] = xpad[:, c, off:off + H * PW]
    for c in range(C):
        g, w = _row_of(c, 9)
        rep[:, g, w, :] = np.float16(1.0)
        for j in (10, 11):
            g, w = _row_of(c, j)
            rep[:, g, w, :] = np.float16(4096.0)
    # repack each pre-shifted 66-col row window to the dense 64 cols the
    # matmul actually reads: -3% upload bytes on the feed-bound input path
    rep = np.ascontiguousarray(
        rep.reshape(B, NG, KC, H, PW)[:, :, :, :, :W]).reshape(
        B, NG, KC, H * W)
    parts = []
    for pb in range(NPB):
        r0, nr = BLK_R0[pb], BLK_ROWS[pb]
        sl = rep[:, :, :, r0 * W:(r0 + nr) * W]             # [B, NG, KC, w]
        parts.append(sl.transpose(0, 2, 1, 3).reshape(B, KC, NG * nr * W))
    return parts


def _split_multiwaits(nc):
    """This container's walrus allows one sync-wait per instruction; move
    extras onto preceding same-engine NoOps."""
    for bb in nc.main_func.blocks:
        insts = bb.instructions
        i = 0
        while i < len(insts):
            ins = insts[i]
            si = getattr(ins, "sync_info", None)
            if si is not None and si.on_wait is not None and len(si.on_wait) > 1:
                waits = list(si.on_wait)
                nops = []
                for j, w in enumerate(waits[:-1]):
                    nop = mybir.InstNoOp(name=f"{ins.name}-wsplit{j}", ins=[], outs=[])
                    nop.engine = ins.engine
                    nop.sync_info = mybir.SyncInfo(on_wait=[w], on_update=[])
                    nops.append(nop)
                si.on_wait = [waits[-1]]
                ins.sync_info = si
                for j, nop in enumerate(nops):
                    insts.insert(i + j, nop)
                i += len(nops)
            i += 1


def _build_nc():
    # boot carries only the KC=120 live partitions: -6% transfer time on
    # the critical start path (real work is gated by the boot-DMA sem)
    bootw = WCW + NG * BLK_ROWS[0] * W       # boot: weights + block 0
    xrepw = NG * (H - BLK_ROWS[0]) * W       # xrep dram: blocks 1..NPB-1

    nc = bass.Bass()
    boot_d = nc.dram_tensor("boot", [KC, bootw], F16, kind="ExternalInput")
    gm_d = nc.dram_tensor("gmat", [128, GMW], F8, kind="ExternalInput")
    xrep_d = nc.dram_tensor("xrep", [KC, xrepw], F16, kind="ExternalInput")
    out_d = nc.dram_tensor("out", [OUT, H * W], F16, kind="ExternalOutput")

    with TileContext(nc) as tc:
        with (
            tc.tile_pool(name="const", bufs=1) as cpool,
            tc.tile_pool(name="blk", bufs=1) as bpool,
            tc.tile_pool(name="work", bufs=6) as wpool,
            tc.tile_pool(name="outp", bufs=5) as opool,
            tc.tile_pool(name="psP", bufs=5, space="PSUM") as psP,
            tc.tile_pool(name="psR", bufs=3, space="PSUM") as psR,
        ):
            # warmup: PE p-state ramp on a zero tile
            warm = cpool.tile([128, 128], F16, tag="warm")
            nc.gpsimd.memset(warm[:, :], 0.0)
            warm_ps = psP.tile([128, NDEN], F32, tag="P", name="warmps")
            for wi in range(NWARM):
                nc.tensor.matmul(warm_ps[:, :128], warm[:, :], warm[:, :],
                                 start=True, stop=True)

            boot = cpool.tile([KC, bootw], F16, tag="boot")
            nc.sync.dma_start(out=boot[:, :], in_=boot_d[:, :])
            gm = cpool.tile([128, GMW], F8, tag="gm")
            nc.sync.dma_start(out=gm[:, :], in_=gm_d[:, :])
            # fp16 gmat for the tail path: derived on-device from the fp8
            # copy (pow2 values, both dtypes exact) instead of riding boot
            gm16 = cpool.tile([128, 64], F16, tag="gm16")
            nc.vector.tensor_copy(
                AP(tensor=gm16.tensor, offset=gm16.offset,
                   ap=[[64, 128], [32, 2], [1, 32]]),
                AP(tensor=gm.tensor, offset=gm.offset,
                   ap=[[GMW, 128], [64, 2], [1, 32]]))

            blk = [None] * NPB
            off_x = 0
            for pb in range(1, NPB):
                w = NG * BLK_ROWS[pb] * W
                blk[pb] = bpool.tile([KC, w], F16, tag=f"blk{pb}",
                                     name=f"blk{pb}")
                src = AP(tensor=xrep_d, offset=off_x, ap=[[xrepw, KC], [1, w]])
                dst = AP(tensor=blk[pb].tensor, offset=blk[pb].offset,
                         ap=[[w, KC], [1, w]])
                nc.sync.dma_start(out=dst, in_=src)
                off_x += w

            def stage1(pb):
                nr = BLK_ROWS[pb]
                n = nr * W
                gw = nr * W
                P = [psP.tile([128, NDEN], F32, tag="P", name=f"P{pb}_{m}")
                     for m in range(2)]
                # last two blocks run m-outer so P0's chain (and its copy)
                # completes 4 matmuls earlier, hiding copy latency in the
                # tail; earlier blocks stay m-inner
                gm_order = [(g, m) for m in range(2) for g in range(NG)]
                for g, m in gm_order:
                    if pb == 0:
                        t = boot
                        off, pitch = boot.offset + WCW + g * gw, bootw
                    else:
                        t = blk[pb]
                        off, pitch = t.offset + g * gw, NG * gw
                    rhs = AP(tensor=t.tensor, offset=off,
                             ap=[[pitch, KC], [1, n]])
                    lt = boot[0:KC, (g * 2 + m) * 128:(g * 2 + m + 1) * 128]
                    nc.tensor.matmul(P[m][:, :n], lt, rhs,
                                     start=(g == 0), stop=(g == NG - 1))
                if pb >= NPB - F16_LAST:
                    # light tail path: fp16 A (ints <= 2048 exact), 2 copies
                    A = wpool.tile([128, 2 * NDEN], F16, tag="A16",
                                   name=f"A{pb}")
                    nc.scalar.copy(A[:, 0:n], P[0][:, :n])
                    nc.vector.tensor_copy(A[:, NDEN:NDEN + n], P[1][:, :n])
                    return A
                # A: fp8 tile [Ah0|Ah1|Al0|Al1]. P values are channel-summed
                # ints (|P| <~ 150): Dekker-split P = Ah + Al, both exact in
                # e4m3 (RNE residual of an int <= 256 is an int <= 8).
                A = wpool.tile([128, 4 * NDEN], F8, tag="A", name=f"A{pb}")
                nc.scalar.copy(A[:, 0:n], P[0][:, :n])
                nc.vector.scalar_tensor_tensor(
                    A[:, 2 * NDEN:2 * NDEN + n], P[0][:, :n], 0.0, A[:, 0:n],
                    mybir.AluOpType.add, mybir.AluOpType.subtract)
                nc.scalar.copy(A[:, NDEN:NDEN + n], P[1][:, :n])
                nc.vector.scalar_tensor_tensor(
                    A[:, 3 * NDEN:3 * NDEN + n], P[1][:, :n], 0.0,
                    A[:, NDEN:NDEN + n],
                    mybir.AluOpType.add, mybir.AluOpType.subtract)
                return A

            osb = None
            osb_off = 0
            osb_pb0 = 0
            pair_start = (list(PAIR_START) if PAIR_START is not None else
                          [True] + [(pb % 2) == (NPB % 2)
                                    for pb in range(1, NPB)])

            def stage2(pb, A):
                nonlocal osb, osb_off, osb_pb0
                nr = BLK_ROWS[pb]
                n = nr * W
                R = psR.tile([32, NDEN], F32, tag="R", name=f"R{pb}")
                if pb >= NPB - F16_LAST:
                    # fp16 A: two plain matmuls with the derived fp16 gmat
                    nc.tensor.matmul(R[:, :n], gm16[:, 0:32],
                                     A[:, 0:n], start=True, stop=False)
                    nc.tensor.matmul(R[:, :n], gm16[:, 32:64],
                                     A[:, NDEN:NDEN + n],
                                     start=False, stop=True)
                elif pb >= NPB - SPLIT_S2_LAST:
                    # four plain fp8 matmuls: each needs only one A quarter
                    for lo, ao in ((0, 0), (0, 2 * NDEN), (64, NDEN),
                                   (64, 3 * NDEN)):
                        nc.tensor.matmul(R[:, :n], gm[:, lo:lo + 32],
                                         A[:, ao:ao + n],
                                         start=(ao == 0), stop=(lo == 64 and
                                                                ao >= 3 * NDEN))
                else:
                    # two DoubleRow matmuls; pair dim = (Ah, Al) per m-half,
                    # so the m0 matmul only needs the m0 copies
                    for m in range(2):
                        lt = AP(tensor=gm.tensor, offset=gm.offset + 64 * m,
                                ap=[[GMW, 128], [32, 2], [1, 32]])
                        rhs = AP(tensor=A.tensor, offset=A.offset + m * NDEN,
                                 ap=[[4 * NDEN, 128], [2 * NDEN, 2], [1, n]])
                        nc.tensor.matmul(R[:, :n], lt, rhs, start=(m == 0),
                                         stop=(m == 1),
                                         perf_mode=mybir.MatmulPerfMode.DoubleRow)
                if pair_start[pb]:
                    osb = opool.tile([32, 2 * NDEN], F16, tag="osb",
                                     name=f"osb{pb}")
                    osb_off = 0
                    osb_pb0 = pb
                out_ap = AP(tensor=osb.tensor, offset=osb.offset + osb_off,
                            ap=[[2 * NDEN, 32], [1, n]])
                if R_ON_DVE(pb):
                    nc.vector.tensor_copy(out_ap, R[:, :n])
                else:
                    nc.scalar.copy(out_ap, R[:, :n])
                osb_off += n
                if pb == NPB - 1 or pair_start[pb + 1]:
                    pb0 = osb_pb0
                    nrows = sum(BLK_ROWS[pb0:pb + 1])
                    dst = AP(tensor=out_d, offset=BLK_R0[pb0] * W,
                             ap=[[H * W, OUT], [1, nrows * W]])
                    src = AP(tensor=osb.tensor, offset=osb.offset,
                             ap=[[2 * NDEN, 32], [1, nrows * W]])
                    nc.sync.dma_start(out=dst, in_=src)

            def lag_of(q):
                if q >= NPB - F16_LAST:
                    return 1
                # last two Dekker blocks: shorter lag so their R copies
                # free psR banks before the tail stage-2s need them (the
                # scheduler reorders around waits, so emitting earlier
                # lets them run at dep-ready time)
                return 2 if q >= NPB - F16_LAST - 2 else S2_LAG

            As = []
            nxt = 0
            for pb in range(NPB):
                As.append(stage1(pb))
                while nxt < NPB and nxt <= pb - lag_of(nxt):
                    stage2(nxt, As[nxt])
                    nxt += 1
            while nxt < NPB:
                stage2(nxt, As[nxt])
                nxt += 1

    _split_multiwaits(nc)
    return nc


def kernel(x, weight, bias):
    x = np.asarray(x, np.float32)
    weight = np.asarray(weight, np.float32)
    bias = np.asarray(bias, np.float32)

    parts = _build_xrep(x)
    wc, gmat = _prep_weights(weight, bias)
    bootw = WCW + NG * BLK_ROWS[0] * W

    if "nc" not in _cache:
        _cache["nc"] = _build_nc()
    nc = _cache["nc"]

    in_maps = []
    for b in range(B):
        boot = np.zeros((KC, bootw), np.float16)
        boot[:, :WCW] = wc
        boot[:, WCW:] = parts[0][b]
        xr = np.concatenate([p[b] for p in parts[1:]], axis=1)
        in_maps.append({"boot": boot, "gmat": gmat, "xrep": xr})
    res = run_bass_kernel_spmd(nc, in_maps, core_ids=list(range(B)))
    out = np.stack([np.asarray(r["out"], np.float32) for r in res.results])
    return (out * SCALE).reshape(B, OUT, H, W).astype(np.float32)
